# revision 8
# baseline (speedup 1.0000x reference)
"""Deformable depthwise conv (DConv) Trainium2 kernel — V4.

V3 -> V4 changes, all aimed at the three measured bottlenecks (Pool 235us
busy incl. 25us of gating-product TTs, 49us serial preamble head, 17us
tail):

- Quarter-image pipelining: the image is processed in 4 row-blocks (qb) of
  16 rows / 1024 px. Each qb's preamble (conv, fields, transposes, index
  build, gating rows) is emitted between the previous blocks' main loops,
  so the first gather starts after ~1/4 of the preamble and the tail
  shrinks to one quarter-block drain.
- Pool engine runs ONLY ap_gathers (205us): the gating-product
  tensor_tensors moved to DVE, and gathers are batched 3 (tap,yc)-units
  per ISA call (num_idxs=3072).
- DVE gating multiplies batched to one TT per gather call ([128, 6144]
  bf16, 2x mode) to amortize instruction overheads.
- Incremental odd-plane build per 16-row load so gathers never wait on
  the full-image shifted copy.

Same math as V3: offsets via 18-row conv on PE; bilinear fields with the
+7.5 round-to-floor bias; pair-packed f32 gathers from padded even/odd
bf16 planes; per-pixel gating rows broadcast via DMA; depthwise weights
applied by PE as diag-matmul accumulation over (tap, y-corner, x-corner)
into PSUM.
"""

import os
import numpy as np

import concourse.bass as bass
import concourse.bacc as bacc
import concourse.mybir as mybir
import concourse.tile as tile

f32 = mybir.dt.float32
bf16 = mybir.dt.bfloat16
i32 = mybir.dt.int32
i16 = mybir.dt.int16

B, C, H, W = 8, 256, 64, 64
HW = H * W            # 4096
PAD = 2
PW = W + 2 * PAD      # 68
NPIX = PW * PW        # 4624
NPIXA = 4736          # 37*128, aligned alloc for the padded planes
HALF = NPIXA // 2     # odd-plane offset in f32-pair units
KK = 9                # 3x3 taps
NCORES = 8
FBIAS = 7.5           # bias so HW round-to-nearest cast == floor+8

QB = 4                # row-blocks
PXQ = HW // QB        # 1024 px per block
GQ = 64               # 16-px groups per block (W18 columns)
UPC = 3               # (tap,yc) units per gather call
NCALL = 18 // UPC     # 6 gather calls per (qb, h)
NIDX = UPC * PXQ      # 3072 f32-pair gathers per call

AF = mybir.ActivationFunctionType
ALU = mybir.AluOpType


def _build_nc():
    nc = bacc.Bacc("TRN2", target_bir_lowering=False, debug=False,
                   num_devices=NCORES)
    x_d = nc.dram_tensor("x", [C, H, W], f32, kind="ExternalInput")
    wo_d = nc.dram_tensor("wo", [2, 128, KK, 18], bf16, kind="ExternalInput")
    wdiag_d = nc.dram_tensor("wdiag", [128, 18, 128], bf16,
                             kind="ExternalInput")
    base_d = nc.dram_tensor("base", [18, HW], bf16, kind="ExternalInput")
    ident_d = nc.dram_tensor("ident", [32, 32], bf16, kind="ExternalInput")
    out_d = nc.dram_tensor("out", [C, H, W], f32, kind="ExternalOutput")
    gat_d = nc.dram_tensor("gat_scratch", [QB, 18 * 2 * PXQ], bf16,
                           kind="Internal")

    with tile.TileContext(nc) as tc:
        _kernel(tc, out_d, x_d, wo_d, wdiag_d, base_d, ident_d, gat_d)
    nc.compile()
    return nc


def _kernel(tc, out_d, x_d, wo_d, wdiag_d, base_d, ident_d, gat_d):
    nc = tc.nc

    with tc.tile_pool(name="persist", bufs=1) as persist:
        # ---------------- constants ----------------
        wo_sb = [persist.tile([128, KK, 18], bf16, name=f"wo{h}",
                              tag=f"wo{h}") for h in range(2)]
        for h in range(2):
            nc.sync.dma_start(wo_sb[h][:], wo_d[h])
        wdiag = persist.tile([128, 18, 128], bf16, tag="wdiag")
        nc.sync.dma_start(wdiag[:], wdiag_d[:])
        ident = persist.tile([32, 32], bf16, tag="ident")
        nc.sync.dma_start(ident[:], ident_d[:])

        xp2 = [persist.tile([128, 2, NPIXA], bf16, name=f"xp2_{h}",
                            tag=f"xp2_{h}") for h in range(2)]
        idxR = persist.tile([128, QB, KK, 2, GQ], i16, tag="idxR")
        cpool_cm = tc.tile_pool(name="cpool", bufs=1)
        cpool = cpool_cm.__enter__()
        base = cpool.tile([18, HW], bf16, tag="base")
        nc.sync.dma_start(base[:], base_d[:])

        ldpool_cm = tc.tile_pool(name="ldpool", bufs=2)
        ldpool = ldpool_cm.__enter__()
        for h in range(2):
            # zero both planes once; interior rows overwritten below
            nc.scalar.memzero(xp2[h][:, 0, :])
            nc.scalar.memzero(xp2[h][:, 1, :])

        def load_q(q):
            # contiguous DMA chunk into staging (16KB runs, full DMA rate),
            # then strided bf16 convert into the padded even plane, then
            # the freshly-available span of the odd (shift-by-1) plane.
            for h in range(2):
                xs = ldpool.tile([128, 1024], f32, tag=f"xs{h}")
                nc.sync.dma_start(
                    xs[:],
                    x_d[128 * h:128 * (h + 1),
                        16 * q:16 * (q + 1)].rearrange("c y x -> c (y x)"),
                )
                dst = xp2[h][:, 0, 0:NPIX].rearrange(
                    "p (y x) -> p y x", y=PW, x=PW)[
                    :, PAD + 16 * q:PAD + 16 * (q + 1), PAD:PAD + W]
                nc.scalar.activation(
                    dst,
                    xs[:].rearrange("p (y x) -> p y x", y=16, x=W),
                    AF.Copy)
                lo = 0 if q == 0 else PW * (16 * q + PAD) - 1
                hi = NPIXA - 1 if q == 3 else PW * (16 * q + 16 + PAD) - 1
                nc.scalar.activation(xp2[h][:, 1, lo:hi],
                                     xp2[h][:, 0, lo + 1:hi + 1], AF.Copy)

        fpool_cm = tc.tile_pool(name="fpool", bufs=1)
        fpool = fpool_cm.__enter__()
        fS = fpool.tile([18, HW], bf16, tag="fS")
        omfS = fpool.tile([18, HW], bf16, tag="omfS")
        W18 = fpool.tile([16, 256, 18], bf16, tag="W18")

        psc_cm = tc.tile_pool(name="psc", bufs=2, space=bass.MemorySpace.PSUM)
        psc = psc_cm.__enter__()
        psw_cm = tc.tile_pool(name="psw", bufs=2, space=bass.MemorySpace.PSUM)
        psw = psw_cm.__enter__()
        ftmp_cm = tc.tile_pool(name="ftmp", bufs=2)
        ftmp = ftmp_cm.__enter__()
        ipool_cm = tc.tile_pool(name="ipool", bufs=1)
        ipool = ipool_cm.__enter__()
        gatp_cm = tc.tile_pool(name="gatp", bufs=1)
        gatp = gatp_cm.__enter__()

        xpb3 = [xp2[h][:, 0, 0:NPIX].rearrange("p (y x) -> p y x", y=PW, x=PW)
                for h in range(2)]

        def conv_chunk(n):
            oc = ftmp.tile([18, 512], f32, tag="offs")
            pt = psc.tile([18, 512], f32, tag="convps")
            first = True
            for t in range(KK):
                dy, dx = t // 3, t % 3
                for h in range(2):
                    rhs = xpb3[h][:, (dy + 1) + 8 * n:(dy + 1) + 8 * n + 8,
                                  (dx + 1):(dx + 1) + W]
                    nc.tensor.matmul(pt[:], wo_sb[h][:, t, :], rhs,
                                     start=first,
                                     stop=(t == KK - 1 and h == 1))
                    first = False
            nc.scalar.activation(oc[:], pt[:], AF.Copy)
            return oc

        def fields_chunk(n, oc):
            cs = slice(512 * n, 512 * (n + 1))
            nfi = ftmp.tile([18, 512], i32, tag="nfi")
            nf = ftmp.tile([18, 512], f32, tag="nf")
            bS = ftmp.tile([18, 512], bf16, tag="bS")
            fsub = ftmp.tile([18, 512], f32, tag="fsub")
            nc.vector.tensor_add(oc[:], oc[:], base[:, cs])
            nc.vector.tensor_copy(nfi[:], oc[:])
            nc.vector.tensor_copy(nf[:], nfi[:])
            nc.vector.tensor_tensor(fsub[:], oc[:], nf[:],
                                    ALU.subtract)
            nc.scalar.activation(omfS[:, cs], fsub[:], AF.Copy, bias=0.5,
                                 scale=-1.0)
            nc.scalar.activation(fS[:, cs], fsub[:], AF.Copy, bias=0.5,
                                 scale=1.0)
            nc.scalar.activation(bS[:], nf[:], AF.Copy)
            return bS

        def transp_chunk(n, bS):
            for g4 in range(8):
                pw = psw.tile([16, 4, 18], bf16, tag="wrapps")
                for j in range(4):
                    s = 16 * (4 * g4 + j)
                    nc.tensor.transpose(pw[:, j, :], bS[:, s:s + 16],
                                        ident[0:18, 0:18])
                nc.scalar.activation(
                    W18[:, 32 * n + 4 * g4:32 * n + 4 * g4 + 4, :], pw[:],
                    AF.Copy)

        def idx_chain(qb):
            ss = slice(GQ * qb, GQ * (qb + 1))
            ncl = ipool.tile([16, GQ, 18], bf16, tag="ncl")
            FF0 = ipool.tile([16, GQ, KK], f32, tag="FF0")
            ihf = ipool.tile([16, GQ, KK], f32, tag="ihf")
            ihi = ipool.tile([16, GQ, KK], i32, tag="ihi")
            nc.vector.tensor_scalar(ncl[:], W18[:, ss, :], 6.0, 72.0,
                                    ALU.max, ALU.min)
            nc.vector.scalar_tensor_tensor(FF0[:], ncl[:, :, 0:KK], 68.0,
                                           ncl[:, :, KK:18], ALU.mult,
                                           ALU.add)
            nc.vector.tensor_scalar(ihf[:], FF0[:], 0.5, -207.25,
                                    ALU.mult, ALU.add)
            nc.vector.tensor_copy(ihi[:], ihf[:])
            nc.vector.tensor_copy(ihf[:], ihi[:])
            # FF0 <- HALF * (par = FF0 - 2*ih - 414)
            nc.vector.scalar_tensor_tensor(FF0[:], ihf[:], -2.0, FF0[:],
                                           ALU.mult, ALU.add)
            nc.vector.tensor_scalar(FF0[:], FF0[:], -414.0, float(HALF),
                                    ALU.add, ALU.mult)
            # ihf <- idx = ih + HALF*par
            nc.vector.tensor_tensor(ihf[:], ihf[:], FF0[:], ALU.add)
            for yc in range(2):
                dst = idxR[0:16, qb, :, yc, :].rearrange("p k s -> p s k")
                nc.vector.tensor_scalar(dst, ihf[:], 34.0 * yc, 0.0,
                                        ALU.add, ALU.add)
            for st in (16, 32, 64):
                nc.sync.dma_start(
                    idxR[st:2 * st, qb].rearrange("p a b c -> p (a b c)"),
                    idxR[0:st, qb].rearrange("p a b c -> p (a b c)"))

        def gat_build(qb):
            cs = slice(PXQ * qb, PXQ * (qb + 1))
            xx = gatp.tile([KK, 2 * PXQ], bf16, tag="xx")
            nc.sync.dma_start(xx[:, 0:PXQ], omfS[KK:18, cs])
            nc.sync.dma_start(xx[:, PXQ:2 * PXQ], fS[KK:18, cs])
            p1 = gatp.tile([KK, 2 * PXQ], bf16, tag="p1")
            p2 = gatp.tile([KK, 2 * PXQ], bf16, tag="p2")
            p1v = p1[:].rearrange("p (j two) -> p two j", two=2)
            p2v = p2[:].rearrange("p (j two) -> p two j", two=2)
            for dx in range(2):
                wx = xx[:, PXQ * dx:PXQ * (dx + 1)]
                nc.vector.tensor_tensor(p1v[:, dx, :], omfS[0:KK, cs], wx,
                                        ALU.mult)
                nc.vector.tensor_tensor(p2v[:, dx, :], fS[0:KK, cs], wx,
                                        ALU.mult)
            gv = gat_d[qb].rearrange("(k y j) -> k y j", k=KK, y=2)
            nc.sync.dma_start(gv[:, 0, :], p1[:])
            nc.sync.dma_start(gv[:, 1, :], p2[:])

        srcs = [xp2[h][:].rearrange("p t f -> p (t f)").bitcast(
            f32).unsqueeze(2) for h in range(2)]

        pso_cm = tc.tile_pool(name="pso", bufs=1, space=bass.MemorySpace.PSUM)
        pso = pso_cm.__enter__()
        rpool_cm = tc.tile_pool(name="rpool", bufs=2)
        rpool = rpool_cm.__enter__()
        gpool_cm = tc.tile_pool(name="gpool", bufs=2)
        gpool = gpool_cm.__enter__()
        opool_cm = tc.tile_pool(name="opool", bufs=2)
        opool = opool_cm.__enter__()

        def main_block(qb):
            ops = [pso.tile([128, PXQ], f32, name=f"outps{qb}_{h}",
                            tag=f"outps{h}") for h in range(2)]
            reps = []
            for c in range(NCALL):
                rt = rpool.tile([128, 2 * NIDX], bf16, tag="rep")
                nc.sync.dma_start(
                    rt[:],
                    gat_d[qb, 2 * NIDX * c:2 * NIDX * (c + 1)].unsqueeze(
                        0).broadcast_to([128, 2 * NIDX]))
                reps.append(rt)
            idxflat = idxR[:, qb].rearrange("p k y s -> p (k y s)")
            ic = NIDX // 16
            for c in range(NCALL):
                for h in range(2):
                    gt = gpool.tile([128, NIDX, 1], f32, tag="G")
                    nc.gpsimd.ap_gather(
                        gt[:], srcs[h],
                        idxflat[:, ic * c:ic * (c + 1)],
                        channels=128, num_elems=NPIXA, d=1,
                        num_idxs=NIDX)
                    gb = gt[:].rearrange("p f one -> p (f one)").bitcast(bf16)
                    nc.vector.tensor_tensor(gb, gb, reps[c][:], ALU.mult)
                    hj = gb.rearrange("p (u j two) -> p u two j", u=UPC,
                                      two=2)
                    for i in range(UPC):
                        k = (UPC * c + i) // 2
                        for dx in range(2):
                            for m in range(2):
                                ms = slice(512 * m, 512 * (m + 1))
                                nc.tensor.matmul(
                                    ops[h][:, ms],
                                    wdiag[:, 2 * k + h, :],
                                    hj[:, i, dx, ms],
                                    start=(c == 0 and i == 0 and dx == 0),
                                    stop=(c == NCALL - 1 and i == UPC - 1
                                          and dx == 1),
                                )
            for h in range(2):
                osb = opool.tile([128, PXQ], f32, tag="osb")
                nc.scalar.activation(osb[:], ops[h][:], AF.Copy)
                nc.sync.dma_start(
                    out_d[128 * h:128 * (h + 1),
                          16 * qb:16 * (qb + 1)].rearrange(
                        "c y x -> c (y x)"),
                    osb[:])

        def preamble(qb):
            if qb == 0:
                load_q(0)
            if qb < 3:
                load_q(qb + 1)
            oc0 = conv_chunk(2 * qb)
            oc1 = conv_chunk(2 * qb + 1)
            bS0 = fields_chunk(2 * qb, oc0)
            bS1 = fields_chunk(2 * qb + 1, oc1)
            transp_chunk(2 * qb, bS0)
            transp_chunk(2 * qb + 1, bS1)
            idx_chain(qb)
            gat_build(qb)

        # software-pipelined emission: preambles run one block ahead of the
        # main loops so the gather stream starts after ~1/4 of the preamble
        preamble(0)
        preamble(1)
        main_block(0)
        preamble(2)
        main_block(1)
        preamble(3)
        main_block(2)
        main_block(3)

        opool_cm.__exit__(None, None, None)
        gpool_cm.__exit__(None, None, None)
        rpool_cm.__exit__(None, None, None)
        pso_cm.__exit__(None, None, None)
        gatp_cm.__exit__(None, None, None)
        ipool_cm.__exit__(None, None, None)
        ftmp_cm.__exit__(None, None, None)
        psw_cm.__exit__(None, None, None)
        psc_cm.__exit__(None, None, None)
        fpool_cm.__exit__(None, None, None)
        ldpool_cm.__exit__(None, None, None)
        cpool_cm.__exit__(None, None, None)


def _host_inputs(w_offset, w_deform):
    """Build per-core constant inputs (everything except the image)."""
    import ml_dtypes
    wo = np.empty((2, 128, KK, 18), np.float32)
    for h in range(2):
        for t in range(KK):
            ky, kx = t // 3, t % 3
            for m in range(18):
                oc = 2 * m if m < 9 else 2 * (m - 9) + 1
                wo[h, :, t, m] = w_offset[oc, 128 * h:128 * (h + 1), ky, kx]
    wd = w_deform.reshape(C, KK)
    wdiag = np.zeros((128, 18, 128), np.float32)
    for k in range(KK):
        for h in range(2):
            np.fill_diagonal(wdiag[:, 2 * k + h, :],
                             wd[128 * h:128 * (h + 1), k])
    base = np.empty((18, HW), np.float32)
    yy, xx = np.mgrid[0:H, 0:W]
    for k in range(KK):
        ky, kx = k // 3, k % 3
        base[k, :] = (yy + ky - 1).reshape(-1) + FBIAS
        base[9 + k, :] = (xx + kx - 1).reshape(-1) + FBIAS
    ident = np.eye(32, dtype=ml_dtypes.bfloat16)
    return {"wo": wo.astype(ml_dtypes.bfloat16),
            "wdiag": wdiag.astype(ml_dtypes.bfloat16),
            "base": base.astype(ml_dtypes.bfloat16), "ident": ident}


_NC_CACHE = None
LAST_EXEC_NS = None


def kernel(x, w_offset, w_deform):
    global _NC_CACHE
    x = np.asarray(x, np.float32)
    w_offset = np.asarray(w_offset, np.float32)
    w_deform = np.asarray(w_deform, np.float32)

    consts = _host_inputs(w_offset, w_deform)
    in_maps = [dict(consts, x=np.ascontiguousarray(x[i])) for i in range(B)]

    if _NC_CACHE is None:
        _NC_CACHE = _build_nc()
    nc = _NC_CACHE

    from concourse.bass_utils import run_bass_kernel_spmd
    global LAST_EXEC_NS
    trace = bool(os.environ.get("BASS_TRACE"))
    res = run_bass_kernel_spmd(nc, in_maps, core_ids=list(range(NCORES)),
                               trace=trace)
    LAST_EXEC_NS = res.exec_time_ns
    return np.stack([res.results[i]["out"] for i in range(B)], axis=0)


if __name__ == "__main__":
    import jax
    import reference
    cpu = jax.devices("cpu")[0]
    with jax.default_device(cpu):
        jinputs = reference.setup_inputs()
        jexpected = reference.reference(**jinputs)
    inputs = {k: np.asarray(jax.device_get(v)) for k, v in jinputs.items()}
    expected = np.asarray(jax.device_get(jexpected))
    actual = kernel(**inputs)
    rel = np.linalg.norm(actual - expected) / np.linalg.norm(expected)
    print("Relative error:", rel)
    print("max abs diff:", np.abs(actual - expected).max())
    from concourse.timeline_sim import TimelineSim
    print("HW exec time:", round(TimelineSim(_NC_CACHE).simulate()), "ns")


# revision 14
# speedup vs baseline: 1.1195x; 1.1195x over previous
"""Deformable depthwise conv (DConv) Trainium2 kernel — V4.

V3 -> V4 changes, all aimed at the three measured bottlenecks (Pool 235us
busy incl. 25us of gating-product TTs, 49us serial preamble head, 17us
tail):

- Quarter-image pipelining: the image is processed in 4 row-blocks (qb) of
  16 rows / 1024 px. Each qb's preamble (conv, fields, transposes, index
  build, gating rows) is emitted between the previous blocks' main loops,
  so the first gather starts after ~1/4 of the preamble and the tail
  shrinks to one quarter-block drain.
- Pool engine runs ONLY ap_gathers (205us): the gating-product
  tensor_tensors moved to DVE, and gathers are batched 3 (tap,yc)-units
  per ISA call (num_idxs=3072).
- DVE gating multiplies batched to one TT per gather call ([128, 6144]
  bf16, 2x mode) to amortize instruction overheads.
- Incremental odd-plane build per 16-row load so gathers never wait on
  the full-image shifted copy.

Same math as V3: offsets via 18-row conv on PE; bilinear fields with the
+7.5 round-to-floor bias; pair-packed f32 gathers from padded even/odd
bf16 planes; per-pixel gating rows broadcast via DMA; depthwise weights
applied by PE as diag-matmul accumulation over (tap, y-corner, x-corner)
into PSUM.
"""

import os
import numpy as np

import concourse.bass as bass
import concourse.bacc as bacc
import concourse.mybir as mybir
import concourse.tile as tile

f32 = mybir.dt.float32
bf16 = mybir.dt.bfloat16
i32 = mybir.dt.int32
i16 = mybir.dt.int16

B, C, H, W = 8, 256, 64, 64
HW = H * W            # 4096
PAD = 2
PW = W + 2 * PAD      # 68
NPIX = PW * PW        # 4624
SH = 38               # padded rows per half-image source (half1 starts row 30)
NPH = PW * SH         # 2584 elements per half plane
HALFN = NPH // 2      # odd-plane offset in f32-pair units (1292)
H1R = 30              # first padded row of half 1
KK = 9                # 3x3 taps
NCORES = 8
FBIAS = 7.5           # bias so HW round-to-nearest cast == floor+8

QB = 4                # row-blocks
PXQ = HW // QB        # 1024 px per block
GQ = 64               # 16-px groups per block (W18 columns)
UPC = 3               # (tap,yc) units per gather call
NCALL = 18 // UPC     # 6 gather calls per (qb, h)
NIDX = UPC * PXQ      # 3072 f32-pair gathers per call

AF = mybir.ActivationFunctionType
ALU = mybir.AluOpType


def _build_nc():
    nc = bacc.Bacc("TRN2", target_bir_lowering=False, debug=False,
                   num_devices=NCORES)
    x_d = nc.dram_tensor("x", [C, H, W], f32, kind="ExternalInput")
    wo_d = nc.dram_tensor("wo", [2, 128, KK, 18], bf16, kind="ExternalInput")
    wdiag_d = nc.dram_tensor("wdiag", [128, 18, 128], bf16,
                             kind="ExternalInput")
    base_d = nc.dram_tensor("base", [18, HW], bf16, kind="ExternalInput")
    ident_d = nc.dram_tensor("ident", [32, 32], bf16, kind="ExternalInput")
    out_d = nc.dram_tensor("out", [C, H, W], f32, kind="ExternalOutput")
    gat_d = nc.dram_tensor("gat_scratch", [QB, 18 * 2 * PXQ], bf16,
                           kind="Internal")

    with tile.TileContext(nc) as tc:
        _kernel(tc, out_d, x_d, wo_d, wdiag_d, base_d, ident_d, gat_d)
    nc.compile()
    return nc


def _kernel(tc, out_d, x_d, wo_d, wdiag_d, base_d, ident_d, gat_d):
    nc = tc.nc

    with tc.tile_pool(name="persist", bufs=1) as persist:
        # ---------------- constants ----------------
        wo_sb = [persist.tile([128, KK, 18], bf16, name=f"wo{h}",
                              tag=f"wo{h}") for h in range(2)]
        for h in range(2):
            nc.sync.dma_start(wo_sb[h][:], wo_d[h])
        wdiag = persist.tile([128, 18, 128], bf16, tag="wdiag")
        nc.sync.dma_start(wdiag[:], wdiag_d[:])
        ident = persist.tile([32, 32], bf16, tag="ident")
        nc.sync.dma_start(ident[:], ident_d[:])

        xp2 = [[persist.tile([128, 2, NPH], bf16, name=f"xp2_{hf}_{h}",
                             tag=f"xp2_{hf}_{h}") for h in range(2)]
               for hf in range(2)]
        idxR = persist.tile([128, QB, KK, 2, GQ], i16, tag="idxR")
        cpool_cm = tc.tile_pool(name="cpool", bufs=1)
        cpool = cpool_cm.__enter__()
        base = cpool.tile([18, HW], bf16, tag="base")
        nc.sync.dma_start(base[:], base_d[:])

        ldpool_cm = tc.tile_pool(name="ldpool", bufs=2)
        ldpool = ldpool_cm.__enter__()
        for hf in range(2):
            for h in range(2):
                # zero both planes once; interior rows overwritten below
                nc.scalar.memzero(xp2[hf][h][:, 0, :])
                nc.scalar.memzero(xp2[hf][h][:, 1, :])

        def load_q(q):
            # contiguous DMA chunk into staging (16KB runs, full DMA rate),
            # then strided bf16 convert into the padded even plane, then
            # the freshly-available span of the odd (shift-by-1) plane.
            r0, r1 = 16 * q + PAD, 16 * q + 16 + PAD  # padded row span
            for h in range(2):
                xs = ldpool.tile([128, 1024], f32, tag=f"xs{h}")
                nc.sync.dma_start(
                    xs[:],
                    x_d[128 * h:128 * (h + 1),
                        16 * q:16 * (q + 1)].rearrange("c y x -> c (y x)"),
                )
                xsv = xs[:].rearrange("p (y x) -> p y x", y=16, x=W)
                for hf in range(2):
                    base_r = H1R * hf
                    a = max(r0, base_r)
                    b = min(r1, base_r + SH)
                    if a >= b:
                        continue
                    dst = xp2[hf][h][:, 0, :].rearrange(
                        "p (y x) -> p y x", y=SH, x=PW)[
                        :, a - base_r:b - base_r, PAD:PAD + W]
                    nc.scalar.activation(dst, xsv[:, a - r0:b - r0, :],
                                         AF.Copy)
                    # odd plane = even shifted one element, over this span
                    lo = 0 if a == base_r else PW * (a - base_r) - 1
                    hi = (NPH - 1 if b == base_r + SH
                          else PW * (b - base_r) - 1)
                    nc.scalar.activation(xp2[hf][h][:, 1, lo:hi],
                                         xp2[hf][h][:, 0, lo + 1:hi + 1],
                                         AF.Copy)

        fpool_cm = tc.tile_pool(name="fpool", bufs=1)
        fpool = fpool_cm.__enter__()
        fS = fpool.tile([18, HW], bf16, tag="fS")
        omfS = fpool.tile([18, HW], bf16, tag="omfS")
        W18 = fpool.tile([16, 256, 18], bf16, tag="W18")

        psc_cm = tc.tile_pool(name="psc", bufs=2, space=bass.MemorySpace.PSUM)
        psc = psc_cm.__enter__()
        psw_cm = tc.tile_pool(name="psw", bufs=2, space=bass.MemorySpace.PSUM)
        psw = psw_cm.__enter__()
        ftmp_cm = tc.tile_pool(name="ftmp", bufs=2)
        ftmp = ftmp_cm.__enter__()
        ipool_cm = tc.tile_pool(name="ipool", bufs=1)
        ipool = ipool_cm.__enter__()
        gatp_cm = tc.tile_pool(name="gatp", bufs=1)
        gatp = gatp_cm.__enter__()

        xpb3 = [[xp2[hf][h][:, 0, :].rearrange("p (y x) -> p y x",
                                               y=SH, x=PW)
                 for h in range(2)] for hf in range(2)]

        def conv_chunk(n):
            oc = ftmp.tile([18, 512], f32, tag="offs")
            pt = psc.tile([18, 512], f32, tag="convps")
            first = True
            hf = 0 if n < 4 else 1
            for t in range(KK):
                dy, dx = t // 3, t % 3
                for h in range(2):
                    r = (dy + 1) + 8 * n - H1R * hf
                    rhs = xpb3[hf][h][:, r:r + 8,
                                      (dx + 1):(dx + 1) + W]
                    nc.tensor.matmul(pt[:], wo_sb[h][:, t, :], rhs,
                                     start=first,
                                     stop=(t == KK - 1 and h == 1))
                    first = False
            nc.scalar.activation(oc[:], pt[:], AF.Copy)
            return oc

        def fields_chunk(n, oc):
            cs = slice(512 * n, 512 * (n + 1))
            nfi = ftmp.tile([18, 512], i32, tag="nfi")
            nf = ftmp.tile([18, 512], f32, tag="nf")
            bS = ftmp.tile([18, 512], bf16, tag="bS")
            fsub = ftmp.tile([18, 512], f32, tag="fsub")
            nc.vector.tensor_add(oc[:], oc[:], base[:, cs])
            nc.vector.tensor_copy(nfi[:], oc[:])
            nc.vector.tensor_copy(nf[:], nfi[:])
            nc.vector.tensor_tensor(fsub[:], oc[:], nf[:],
                                    ALU.subtract)
            nc.scalar.activation(omfS[:, cs], fsub[:], AF.Copy, bias=0.5,
                                 scale=-1.0)
            nc.scalar.activation(fS[:, cs], fsub[:], AF.Copy, bias=0.5,
                                 scale=1.0)
            nc.scalar.activation(bS[:], nf[:], AF.Copy)
            return bS

        def transp_chunk(n, bS):
            for g4 in range(8):
                pw = psw.tile([16, 4, 18], bf16, tag="wrapps")
                for j in range(4):
                    s = 16 * (4 * g4 + j)
                    nc.tensor.transpose(pw[:, j, :], bS[:, s:s + 16],
                                        ident[0:18, 0:18])
                nc.scalar.activation(
                    W18[:, 32 * n + 4 * g4:32 * n + 4 * g4 + 4, :], pw[:],
                    AF.Copy)

        def idx_chain(qb):
            ss = slice(GQ * qb, GQ * (qb + 1))
            ncl = ipool.tile([16, GQ, 18], bf16, tag="ncl")
            FF0 = ipool.tile([16, GQ, KK], f32, tag="FF0")
            ihf = ipool.tile([16, GQ, KK], f32, tag="ihf")
            ihi = ipool.tile([16, GQ, KK], i32, tag="ihi")
            b1 = 1 if qb >= 2 else 0
            nc.vector.tensor_scalar(ncl[:], W18[:, ss, :], 6.0, 72.0,
                                    ALU.max, ALU.min)
            # per-half y clamp keeps reads inside this half's 38 rows; the
            # legit sample range (|off|<2.5 after the global pad design)
            # stays strictly inside these bounds
            ylo, yhi = (6.0, 42.0) if not b1 else (36.0, 72.0)
            nc.vector.tensor_scalar(ncl[:, :, 0:KK], ncl[:, :, 0:KK],
                                    ylo, yhi, ALU.max, ALU.min)
            nc.vector.scalar_tensor_tensor(FF0[:], ncl[:, :, 0:KK], 68.0,
                                           ncl[:, :, KK:18], ALU.mult,
                                           ALU.add)
            nc.vector.tensor_scalar(ihf[:], FF0[:], 0.5,
                                    -207.25 - 1020.0 * b1,
                                    ALU.mult, ALU.add)
            nc.vector.tensor_copy(ihi[:], ihf[:])
            nc.vector.tensor_copy(ihf[:], ihi[:])
            # FF0 <- HALFN * (par = FF0 - 2*ih - 414 - 2040*b1)
            nc.vector.scalar_tensor_tensor(FF0[:], ihf[:], -2.0, FF0[:],
                                           ALU.mult, ALU.add)
            nc.vector.tensor_scalar(FF0[:], FF0[:], -414.0 - 2040.0 * b1,
                                    float(HALFN), ALU.add, ALU.mult)
            # ihf <- idx = ih + HALF*par
            nc.vector.tensor_tensor(ihf[:], ihf[:], FF0[:], ALU.add)
            for yc in range(2):
                dst = idxR[0:16, qb, :, yc, :].rearrange("p k s -> p s k")
                nc.vector.tensor_scalar(dst, ihf[:], 34.0 * yc, 0.0,
                                        ALU.add, ALU.add)
            for st in (16, 32, 64):
                nc.sync.dma_start(
                    idxR[st:2 * st, qb].rearrange("p a b c -> p (a b c)"),
                    idxR[0:st, qb].rearrange("p a b c -> p (a b c)"))

        def gat_build(qb):
            cs = slice(PXQ * qb, PXQ * (qb + 1))
            xx = gatp.tile([KK, 2 * PXQ], bf16, tag="xx")
            nc.sync.dma_start(xx[:, 0:PXQ], omfS[KK:18, cs])
            nc.sync.dma_start(xx[:, PXQ:2 * PXQ], fS[KK:18, cs])
            p1 = gatp.tile([KK, 2 * PXQ], bf16, tag="p1")
            p2 = gatp.tile([KK, 2 * PXQ], bf16, tag="p2")
            p1v = p1[:].rearrange("p (j two) -> p two j", two=2)
            p2v = p2[:].rearrange("p (j two) -> p two j", two=2)
            for dx in range(2):
                wx = xx[:, PXQ * dx:PXQ * (dx + 1)]
                nc.vector.tensor_tensor(p1v[:, dx, :], omfS[0:KK, cs], wx,
                                        ALU.mult)
                nc.vector.tensor_tensor(p2v[:, dx, :], fS[0:KK, cs], wx,
                                        ALU.mult)
            gv = gat_d[qb].rearrange("(k y j) -> k y j", k=KK, y=2)
            nc.sync.dma_start(gv[:, 0, :], p1[:])
            nc.sync.dma_start(gv[:, 1, :], p2[:])

        srcs = [[xp2[hf][h][:].rearrange("p t f -> p (t f)").bitcast(
            f32).unsqueeze(2) for h in range(2)] for hf in range(2)]

        pso_cm = tc.tile_pool(name="pso", bufs=1, space=bass.MemorySpace.PSUM)
        pso = pso_cm.__enter__()
        rpool_cm = tc.tile_pool(name="rpool", bufs=2)
        rpool = rpool_cm.__enter__()
        gpool_cm = tc.tile_pool(name="gpool", bufs=2)
        gpool = gpool_cm.__enter__()
        opool_cm = tc.tile_pool(name="opool", bufs=2)
        opool = opool_cm.__enter__()

        def main_block(qb, pre_steps):
            ops = [pso.tile([128, PXQ], f32, name=f"outps{qb}_{h}",
                            tag=f"outps{h}") for h in range(2)]
            reps = []
            for c in range(NCALL):
                rt = rpool.tile([128, 2 * NIDX], bf16, tag="rep")
                nc.sync.dma_start(
                    rt[:],
                    gat_d[qb, 2 * NIDX * c:2 * NIDX * (c + 1)].unsqueeze(
                        0).broadcast_to([128, 2 * NIDX]))
                reps.append(rt)
            idxflat = idxR[:, qb].rearrange("p k y s -> p (k y s)")
            ic = NIDX // 16
            for c in range(NCALL):
                for h in range(2):
                    gt = gpool.tile([128, NIDX, 1], f32, tag="G")
                    nc.gpsimd.ap_gather(
                        gt[:], srcs[qb // 2][h],
                        idxflat[:, ic * c:ic * (c + 1)],
                        channels=128, num_elems=NPH, d=1,
                        num_idxs=NIDX)
                    gb = gt[:].rearrange("p f one -> p (f one)").bitcast(bf16)
                    nc.vector.tensor_tensor(gb, gb, reps[c][:], ALU.mult)
                    hj = gb.rearrange("p (u j two) -> p u two j", u=UPC,
                                      two=2)
                    for i in range(UPC):
                        k = (UPC * c + i) // 2
                        for dx in range(2):
                            for m in range(2):
                                ms = slice(512 * m, 512 * (m + 1))
                                nc.tensor.matmul(
                                    ops[h][:, ms],
                                    wdiag[:, 2 * k + h, :],
                                    hj[:, i, dx, ms],
                                    start=(c == 0 and i == 0 and dx == 0),
                                    stop=(c == NCALL - 1 and i == UPC - 1
                                          and dx == 1),
                                )
                # interleave next block's preamble steps between gather
                # calls so the DVE queue alternates gating TTs with
                # preamble work and the Pool never waits long on gt bufs
                for step in pre_steps.pop(c, []):
                    step()
            for h in range(2):
                osb = opool.tile([128, PXQ], f32, tag="osb")
                nc.scalar.activation(osb[:], ops[h][:], AF.Copy)
                nc.sync.dma_start(
                    out_d[128 * h:128 * (h + 1),
                          16 * qb:16 * (qb + 1)].rearrange(
                        "c y x -> c (y x)"),
                    osb[:])

        def preamble_steps(qb):
            """Next block's preamble as thunks keyed by gather-call slot."""
            st = {}

            def s0():
                if qb == 0:
                    load_q(0)
                if qb < 3:
                    load_q(qb + 1)
                st["oc0"] = conv_chunk(2 * qb)

            def s1():
                st["oc1"] = conv_chunk(2 * qb + 1)
                st["bS0"] = fields_chunk(2 * qb, st["oc0"])

            def s2():
                st["bS1"] = fields_chunk(2 * qb + 1, st["oc1"])
                transp_chunk(2 * qb, st["bS0"])

            def s3():
                transp_chunk(2 * qb + 1, st["bS1"])

            def s4():
                idx_chain(qb)

            def s5():
                gat_build(qb)

            return {0: [s0], 1: [s1], 2: [s2], 3: [s3], 4: [s4], 5: [s5]}

        # software-pipelined emission: block qb+1's preamble is emitted
        # piecewise between block qb's gather calls
        for step in preamble_steps(0).values():
            for s in step:
                s()
        main_block(0, preamble_steps(1))
        main_block(1, preamble_steps(2))
        main_block(2, preamble_steps(3))
        main_block(3, {})

        opool_cm.__exit__(None, None, None)
        gpool_cm.__exit__(None, None, None)
        rpool_cm.__exit__(None, None, None)
        pso_cm.__exit__(None, None, None)
        gatp_cm.__exit__(None, None, None)
        ipool_cm.__exit__(None, None, None)
        ftmp_cm.__exit__(None, None, None)
        psw_cm.__exit__(None, None, None)
        psc_cm.__exit__(None, None, None)
        fpool_cm.__exit__(None, None, None)
        ldpool_cm.__exit__(None, None, None)
        cpool_cm.__exit__(None, None, None)


def _host_inputs(w_offset, w_deform):
    """Build per-core constant inputs (everything except the image)."""
    import ml_dtypes
    wo = np.empty((2, 128, KK, 18), np.float32)
    for h in range(2):
        for t in range(KK):
            ky, kx = t // 3, t % 3
            for m in range(18):
                oc = 2 * m if m < 9 else 2 * (m - 9) + 1
                wo[h, :, t, m] = w_offset[oc, 128 * h:128 * (h + 1), ky, kx]
    wd = w_deform.reshape(C, KK)
    wdiag = np.zeros((128, 18, 128), np.float32)
    for k in range(KK):
        for h in range(2):
            np.fill_diagonal(wdiag[:, 2 * k + h, :],
                             wd[128 * h:128 * (h + 1), k])
    base = np.empty((18, HW), np.float32)
    yy, xx = np.mgrid[0:H, 0:W]
    for k in range(KK):
        ky, kx = k // 3, k % 3
        base[k, :] = (yy + ky - 1).reshape(-1) + FBIAS
        base[9 + k, :] = (xx + kx - 1).reshape(-1) + FBIAS
    ident = np.eye(32, dtype=ml_dtypes.bfloat16)
    return {"wo": wo.astype(ml_dtypes.bfloat16),
            "wdiag": wdiag.astype(ml_dtypes.bfloat16),
            "base": base.astype(ml_dtypes.bfloat16), "ident": ident}


_NC_CACHE = None
LAST_EXEC_NS = None


def kernel(x, w_offset, w_deform):
    global _NC_CACHE
    x = np.asarray(x, np.float32)
    w_offset = np.asarray(w_offset, np.float32)
    w_deform = np.asarray(w_deform, np.float32)

    consts = _host_inputs(w_offset, w_deform)
    in_maps = [dict(consts, x=np.ascontiguousarray(x[i])) for i in range(B)]

    if _NC_CACHE is None:
        _NC_CACHE = _build_nc()
    nc = _NC_CACHE

    from concourse.bass_utils import run_bass_kernel_spmd
    global LAST_EXEC_NS
    trace = bool(os.environ.get("BASS_TRACE"))
    res = run_bass_kernel_spmd(nc, in_maps, core_ids=list(range(NCORES)),
                               trace=trace)
    LAST_EXEC_NS = res.exec_time_ns
    return np.stack([res.results[i]["out"] for i in range(B)], axis=0)


if __name__ == "__main__":
    import jax
    import reference
    cpu = jax.devices("cpu")[0]
    with jax.default_device(cpu):
        jinputs = reference.setup_inputs()
        jexpected = reference.reference(**jinputs)
    inputs = {k: np.asarray(jax.device_get(v)) for k, v in jinputs.items()}
    expected = np.asarray(jax.device_get(jexpected))
    actual = kernel(**inputs)
    rel = np.linalg.norm(actual - expected) / np.linalg.norm(expected)
    print("Relative error:", rel)
    print("max abs diff:", np.abs(actual - expected).max())
    from concourse.timeline_sim import TimelineSim
    print("HW exec time:", round(TimelineSim(_NC_CACHE).simulate()), "ns")


# revision 17
# speedup vs baseline: 1.4852x; 1.3267x over previous
"""Deformable depthwise conv (DConv) Trainium2 kernel — V4.

V3 -> V4 changes, all aimed at the three measured bottlenecks (Pool 235us
busy incl. 25us of gating-product TTs, 49us serial preamble head, 17us
tail):

- Quarter-image pipelining: the image is processed in 4 row-blocks (qb) of
  16 rows / 1024 px. Each qb's preamble (conv, fields, transposes, index
  build, gating rows) is emitted between the previous blocks' main loops,
  so the first gather starts after ~1/4 of the preamble and the tail
  shrinks to one quarter-block drain.
- Pool engine runs ONLY ap_gathers (205us): the gating-product
  tensor_tensors moved to DVE, and gathers are batched 3 (tap,yc)-units
  per ISA call (num_idxs=3072).
- DVE gating multiplies batched to one TT per gather call ([128, 6144]
  bf16, 2x mode) to amortize instruction overheads.
- Incremental odd-plane build per 16-row load so gathers never wait on
  the full-image shifted copy.

Same math as V3: offsets via 18-row conv on PE; bilinear fields with the
+7.5 round-to-floor bias; pair-packed f32 gathers from padded even/odd
bf16 planes; per-pixel gating rows broadcast via DMA; depthwise weights
applied by PE as diag-matmul accumulation over (tap, y-corner, x-corner)
into PSUM.
"""

import os
import numpy as np

import concourse.bass as bass
import concourse.bacc as bacc
import concourse.mybir as mybir
import concourse.tile as tile

f32 = mybir.dt.float32
bf16 = mybir.dt.bfloat16
i32 = mybir.dt.int32
i16 = mybir.dt.int16

B, C, H, W = 8, 256, 64, 64
HW = H * W            # 4096
PAD = 2
PW = W + 2 * PAD      # 68
NPIX = PW * PW        # 4624
SH = 38               # padded rows per half-image source (half1 starts row 30)
NPH = PW * SH         # 2584 elements per half plane
HALFN = NPH // 2      # odd-plane offset in f32-pair units (1292)
H1R = 30              # first padded row of half 1
KK = 9                # 3x3 taps
NCORES = 8
FBIAS = 7.5           # bias so HW round-to-nearest cast == floor+8

QB = 4                # row-blocks
PXQ = HW // QB        # 1024 px per block
GQ = 64               # 16-px groups per block (W18 columns)
UPC = 3               # (tap,yc) units per gather call
NCALL = 18 // UPC     # 6 gather calls per (qb, h)
NIDX = UPC * PXQ      # 3072 f32-pair gathers per call

AF = mybir.ActivationFunctionType
ALU = mybir.AluOpType


def _build_nc():
    nc = bacc.Bacc("TRN2", target_bir_lowering=False, debug=False,
                   num_devices=NCORES)
    x_d = nc.dram_tensor("x", [C, H, W], f32, kind="ExternalInput")
    wo_d = nc.dram_tensor("wo", [2, 128, KK, 18], bf16, kind="ExternalInput")
    wdiag_d = nc.dram_tensor("wdiag", [128, 18, 128], bf16,
                             kind="ExternalInput")
    base_d = nc.dram_tensor("base", [18, HW], bf16, kind="ExternalInput")
    ident_d = nc.dram_tensor("ident", [32, 32], bf16, kind="ExternalInput")
    out_d = nc.dram_tensor("out", [C, H, W], f32, kind="ExternalOutput")
    gat_d = nc.dram_tensor("gat_scratch", [QB, 18 * 2 * PXQ], bf16,
                           kind="Internal")

    with tile.TileContext(nc) as tc:
        _kernel(tc, out_d, x_d, wo_d, wdiag_d, base_d, ident_d, gat_d)
    nc.compile()
    return nc


def _kernel(tc, out_d, x_d, wo_d, wdiag_d, base_d, ident_d, gat_d):
    nc = tc.nc

    with tc.tile_pool(name="persist", bufs=1) as persist:
        # ---------------- constants ----------------
        wo_sb = [persist.tile([128, KK, 18], bf16, name=f"wo{h}",
                              tag=f"wo{h}") for h in range(2)]
        for h in range(2):
            nc.sync.dma_start(wo_sb[h][:], wo_d[h])
        wdiag = persist.tile([128, 18, 128], bf16, tag="wdiag")
        nc.sync.dma_start(wdiag[:], wdiag_d[:])
        ident = persist.tile([32, 32], bf16, tag="ident")
        nc.sync.dma_start(ident[:], ident_d[:])

        xp2 = [[persist.tile([128, 2, NPH], bf16, name=f"xp2_{hf}_{h}",
                             tag=f"xp2_{hf}_{h}") for h in range(2)]
               for hf in range(2)]
        idxR = persist.tile([128, QB, KK, 2, GQ], i16, tag="idxR")
        cpool_cm = tc.tile_pool(name="cpool", bufs=1)
        cpool = cpool_cm.__enter__()
        base = cpool.tile([18, HW], bf16, tag="base")
        nc.sync.dma_start(base[:], base_d[:])

        ldpool_cm = tc.tile_pool(name="ldpool", bufs=2)
        ldpool = ldpool_cm.__enter__()
        for hf in range(2):
            for h in range(2):
                # zero both planes once; interior rows overwritten below
                nc.scalar.memzero(xp2[hf][h][:, 0, :])
                nc.scalar.memzero(xp2[hf][h][:, 1, :])

        def load_q(q):
            # contiguous DMA chunk into staging (16KB runs, full DMA rate),
            # then strided bf16 convert into the padded even plane, then
            # the freshly-available span of the odd (shift-by-1) plane.
            r0, r1 = 16 * q + PAD, 16 * q + 16 + PAD  # padded row span
            for h in range(2):
                xs = ldpool.tile([128, 1024], f32, tag=f"xs{h}")
                nc.sync.dma_start(
                    xs[:],
                    x_d[128 * h:128 * (h + 1),
                        16 * q:16 * (q + 1)].rearrange("c y x -> c (y x)"),
                )
                xsv = xs[:].rearrange("p (y x) -> p y x", y=16, x=W)
                for hf in range(2):
                    base_r = H1R * hf
                    a = max(r0, base_r)
                    b = min(r1, base_r + SH)
                    if a >= b:
                        continue
                    dst = xp2[hf][h][:, 0, :].rearrange(
                        "p (y x) -> p y x", y=SH, x=PW)[
                        :, a - base_r:b - base_r, PAD:PAD + W]
                    nc.scalar.activation(dst, xsv[:, a - r0:b - r0, :],
                                         AF.Copy)
                    # odd plane = even shifted one element, over this span
                    lo = 0 if a == base_r else PW * (a - base_r) - 1
                    hi = (NPH - 1 if b == base_r + SH
                          else PW * (b - base_r) - 1)
                    nc.scalar.activation(xp2[hf][h][:, 1, lo:hi],
                                         xp2[hf][h][:, 0, lo + 1:hi + 1],
                                         AF.Copy)

        fpool_cm = tc.tile_pool(name="fpool", bufs=1)
        fpool = fpool_cm.__enter__()
        fS = fpool.tile([18, HW], bf16, tag="fS")
        omfS = fpool.tile([18, HW], bf16, tag="omfS")
        W18 = fpool.tile([16, 256, 18], bf16, tag="W18")

        psc_cm = tc.tile_pool(name="psc", bufs=2, space=bass.MemorySpace.PSUM)
        psc = psc_cm.__enter__()
        psw_cm = tc.tile_pool(name="psw", bufs=2, space=bass.MemorySpace.PSUM)
        psw = psw_cm.__enter__()
        ftmp_cm = tc.tile_pool(name="ftmp", bufs=2)
        ftmp = ftmp_cm.__enter__()
        ipool_cm = tc.tile_pool(name="ipool", bufs=1)
        ipool = ipool_cm.__enter__()
        gatp_cm = tc.tile_pool(name="gatp", bufs=1)
        gatp = gatp_cm.__enter__()

        xpb3 = [[xp2[hf][h][:, 0, :].rearrange("p (y x) -> p y x",
                                               y=SH, x=PW)
                 for h in range(2)] for hf in range(2)]

        def conv_chunk(n):
            oc = ftmp.tile([18, 512], f32, tag="offs")
            pt = psc.tile([18, 512], f32, tag="convps")
            first = True
            hf = 0 if n < 4 else 1
            for t in range(KK):
                dy, dx = t // 3, t % 3
                for h in range(2):
                    r = (dy + 1) + 8 * n - H1R * hf
                    rhs = xpb3[hf][h][:, r:r + 8,
                                      (dx + 1):(dx + 1) + W]
                    nc.tensor.matmul(pt[:], wo_sb[h][:, t, :], rhs,
                                     start=first,
                                     stop=(t == KK - 1 and h == 1))
                    first = False
            nc.scalar.activation(oc[:], pt[:], AF.Copy)
            return oc

        def fields_chunk(n, oc):
            cs = slice(512 * n, 512 * (n + 1))
            nfi = ftmp.tile([18, 512], i32, tag="nfi")
            nf = ftmp.tile([18, 512], f32, tag="nf")
            bS = ftmp.tile([18, 512], bf16, tag="bS")
            fsub = ftmp.tile([18, 512], f32, tag="fsub")
            nc.vector.tensor_add(oc[:], oc[:], base[:, cs])
            nc.vector.tensor_copy(nfi[:], oc[:])
            nc.vector.tensor_copy(nf[:], nfi[:])
            nc.vector.tensor_tensor(fsub[:], oc[:], nf[:],
                                    ALU.subtract)
            nc.scalar.activation(omfS[:, cs], fsub[:], AF.Copy, bias=0.5,
                                 scale=-1.0)
            nc.scalar.activation(fS[:, cs], fsub[:], AF.Copy, bias=0.5,
                                 scale=1.0)
            nc.scalar.activation(bS[:], nf[:], AF.Copy)
            return bS

        def transp_chunk(n, bS):
            for g4 in range(8):
                pw = psw.tile([16, 4, 18], bf16, tag="wrapps")
                for j in range(4):
                    s = 16 * (4 * g4 + j)
                    nc.tensor.transpose(pw[:, j, :], bS[:, s:s + 16],
                                        ident[0:18, 0:18])
                nc.scalar.activation(
                    W18[:, 32 * n + 4 * g4:32 * n + 4 * g4 + 4, :], pw[:],
                    AF.Copy)

        def idx_chain(qb):
            ss = slice(GQ * qb, GQ * (qb + 1))
            ncl = ipool.tile([16, GQ, 18], bf16, tag="ncl")
            FF0 = ipool.tile([16, GQ, KK], f32, tag="FF0")
            ihf = ipool.tile([16, GQ, KK], f32, tag="ihf")
            ihi = ipool.tile([16, GQ, KK], i32, tag="ihi")
            b1 = 1 if qb >= 2 else 0
            nc.vector.tensor_scalar(ncl[:], W18[:, ss, :], 6.0, 72.0,
                                    ALU.max, ALU.min)
            # per-half y clamp keeps reads inside this half's 38 rows; the
            # legit sample range (|off|<2.5 after the global pad design)
            # stays strictly inside these bounds
            ylo, yhi = (6.0, 42.0) if not b1 else (36.0, 72.0)
            nc.vector.tensor_scalar(ncl[:, :, 0:KK], ncl[:, :, 0:KK],
                                    ylo, yhi, ALU.max, ALU.min)
            nc.vector.scalar_tensor_tensor(FF0[:], ncl[:, :, 0:KK], 68.0,
                                           ncl[:, :, KK:18], ALU.mult,
                                           ALU.add)
            nc.vector.tensor_scalar(ihf[:], FF0[:], 0.5,
                                    -207.25 - 1020.0 * b1,
                                    ALU.mult, ALU.add)
            nc.vector.tensor_copy(ihi[:], ihf[:])
            nc.vector.tensor_copy(ihf[:], ihi[:])
            # FF0 <- HALFN * (par = FF0 - 2*ih - 414 - 2040*b1)
            nc.vector.scalar_tensor_tensor(FF0[:], ihf[:], -2.0, FF0[:],
                                           ALU.mult, ALU.add)
            nc.vector.tensor_scalar(FF0[:], FF0[:], -414.0 - 2040.0 * b1,
                                    float(HALFN), ALU.add, ALU.mult)
            # ihf <- idx = ih + HALF*par
            nc.vector.tensor_tensor(ihf[:], ihf[:], FF0[:], ALU.add)
            for yc in range(2):
                dst = idxR[0:16, qb, :, yc, :].rearrange("p k s -> p s k")
                nc.vector.tensor_scalar(dst, ihf[:], 34.0 * yc, 0.0,
                                        ALU.add, ALU.add)
            for st in (16, 32, 64):
                nc.sync.dma_start(
                    idxR[st:2 * st, qb].rearrange("p a b c -> p (a b c)"),
                    idxR[0:st, qb].rearrange("p a b c -> p (a b c)"))

        def gat_build(qb):
            cs = slice(PXQ * qb, PXQ * (qb + 1))
            xx = gatp.tile([KK, 2 * PXQ], bf16, tag="xx")
            nc.sync.dma_start(xx[:, 0:PXQ], omfS[KK:18, cs])
            nc.sync.dma_start(xx[:, PXQ:2 * PXQ], fS[KK:18, cs])
            p1 = gatp.tile([KK, 2 * PXQ], bf16, tag="p1")
            p2 = gatp.tile([KK, 2 * PXQ], bf16, tag="p2")
            p1v = p1[:].rearrange("p (j two) -> p two j", two=2)
            p2v = p2[:].rearrange("p (j two) -> p two j", two=2)
            for dx in range(2):
                wx = xx[:, PXQ * dx:PXQ * (dx + 1)]
                nc.vector.tensor_tensor(p1v[:, dx, :], omfS[0:KK, cs], wx,
                                        ALU.mult)
                nc.vector.tensor_tensor(p2v[:, dx, :], fS[0:KK, cs], wx,
                                        ALU.mult)
            gv = gat_d[qb].rearrange("(k y j) -> k y j", k=KK, y=2)
            nc.sync.dma_start(gv[:, 0, :], p1[:])
            nc.sync.dma_start(gv[:, 1, :], p2[:])

        srcs = [[xp2[hf][h][:].rearrange("p t f -> p (t f)").bitcast(
            f32).unsqueeze(2) for h in range(2)] for hf in range(2)]

        pso_cm = tc.tile_pool(name="pso", bufs=1, space=bass.MemorySpace.PSUM)
        pso = pso_cm.__enter__()
        rpool_cm = tc.tile_pool(name="rpool", bufs=2)
        rpool = rpool_cm.__enter__()
        gpool_cm = tc.tile_pool(name="gpool", bufs=3)
        gpool = gpool_cm.__enter__()
        opool_cm = tc.tile_pool(name="opool", bufs=1)
        opool = opool_cm.__enter__()

        def main_block(qb, pre_steps):
            ops = [pso.tile([128, PXQ], f32, name=f"outps{qb}_{h}",
                            tag=f"outps{h}") for h in range(2)]
            reps = []
            for c in range(NCALL):
                rt = rpool.tile([128, 2 * NIDX], bf16, tag="rep")
                nc.sync.dma_start(
                    rt[:],
                    gat_d[qb, 2 * NIDX * c:2 * NIDX * (c + 1)].unsqueeze(
                        0).broadcast_to([128, 2 * NIDX]))
                reps.append(rt)
            idxflat = idxR[:, qb].rearrange("p k y s -> p (k y s)")
            ic = NIDX // 16
            pend = []

            def flush_mm():
                for c, h, hj in pend:
                    for i in range(UPC):
                        k = (UPC * c + i) // 2
                        for dx in range(2):
                            for m in range(2):
                                ms = slice(512 * m, 512 * (m + 1))
                                nc.tensor.matmul(
                                    ops[h][:, ms],
                                    wdiag[:, 2 * k + h, :],
                                    hj[:, i, dx, ms],
                                    start=(c == 0 and i == 0 and dx == 0),
                                    stop=(c == NCALL - 1 and i == UPC - 1
                                          and dx == 1),
                                )
                pend.clear()

            for c in range(NCALL):
                for h in range(2):
                    gt = gpool.tile([128, NIDX, 1], f32, tag="G")
                    nc.gpsimd.ap_gather(
                        gt[:], srcs[qb // 2][h],
                        idxflat[:, ic * c:ic * (c + 1)],
                        channels=128, num_elems=NPH, d=1,
                        num_idxs=NIDX)
                    gb = gt[:].rearrange("p f one -> p (f one)").bitcast(bf16)
                    nc.vector.tensor_tensor(gb, gb, reps[c][:], ALU.mult)
                    pend.append((c, h, gb.rearrange(
                        "p (u j two) -> p u two j", u=UPC, two=2)))
                # batch both halves' matmuls (2 x 12) into one PE burst
                # so the tensor engine ramps toward full clock
                flush_mm()
                # interleave next block's preamble steps between gather
                # calls so the DVE queue alternates gating TTs with
                # preamble work and the Pool never waits long on gt bufs
                for step in pre_steps.pop(c, []):
                    step()
            flush_mm()
            for h in range(2):
                osb = opool.tile([128, PXQ], f32, tag="osb")
                nc.scalar.activation(osb[:], ops[h][:], AF.Copy)
                nc.sync.dma_start(
                    out_d[128 * h:128 * (h + 1),
                          16 * qb:16 * (qb + 1)].rearrange(
                        "c y x -> c (y x)"),
                    osb[:])

        def preamble_steps(qb):
            """Next block's preamble as thunks keyed by gather-call slot."""
            st = {}

            def s0():
                if qb == 0:
                    load_q(0)
                if qb < 3:
                    load_q(qb + 1)
                st["oc0"] = conv_chunk(2 * qb)

            def s1():
                st["oc1"] = conv_chunk(2 * qb + 1)
                st["bS0"] = fields_chunk(2 * qb, st["oc0"])

            def s2():
                st["bS1"] = fields_chunk(2 * qb + 1, st["oc1"])
                transp_chunk(2 * qb, st["bS0"])

            def s3():
                transp_chunk(2 * qb + 1, st["bS1"])

            def s4():
                idx_chain(qb)

            def s5():
                gat_build(qb)

            return {0: [s0], 1: [s1], 2: [s2], 3: [s3], 4: [s4], 5: [s5]}

        # software-pipelined emission: block qb+1's preamble is emitted
        # piecewise between block qb's gather calls
        for step in preamble_steps(0).values():
            for s in step:
                s()
        main_block(0, preamble_steps(1))
        main_block(1, preamble_steps(2))
        main_block(2, preamble_steps(3))
        main_block(3, {})

        opool_cm.__exit__(None, None, None)
        gpool_cm.__exit__(None, None, None)
        rpool_cm.__exit__(None, None, None)
        pso_cm.__exit__(None, None, None)
        gatp_cm.__exit__(None, None, None)
        ipool_cm.__exit__(None, None, None)
        ftmp_cm.__exit__(None, None, None)
        psw_cm.__exit__(None, None, None)
        psc_cm.__exit__(None, None, None)
        fpool_cm.__exit__(None, None, None)
        ldpool_cm.__exit__(None, None, None)
        cpool_cm.__exit__(None, None, None)


def _host_inputs(w_offset, w_deform):
    """Build per-core constant inputs (everything except the image)."""
    import ml_dtypes
    wo = np.empty((2, 128, KK, 18), np.float32)
    for h in range(2):
        for t in range(KK):
            ky, kx = t // 3, t % 3
            for m in range(18):
                oc = 2 * m if m < 9 else 2 * (m - 9) + 1
                wo[h, :, t, m] = w_offset[oc, 128 * h:128 * (h + 1), ky, kx]
    wd = w_deform.reshape(C, KK)
    wdiag = np.zeros((128, 18, 128), np.float32)
    for k in range(KK):
        for h in range(2):
            np.fill_diagonal(wdiag[:, 2 * k + h, :],
                             wd[128 * h:128 * (h + 1), k])
    base = np.empty((18, HW), np.float32)
    yy, xx = np.mgrid[0:H, 0:W]
    for k in range(KK):
        ky, kx = k // 3, k % 3
        base[k, :] = (yy + ky - 1).reshape(-1) + FBIAS
        base[9 + k, :] = (xx + kx - 1).reshape(-1) + FBIAS
    ident = np.eye(32, dtype=ml_dtypes.bfloat16)
    return {"wo": wo.astype(ml_dtypes.bfloat16),
            "wdiag": wdiag.astype(ml_dtypes.bfloat16),
            "base": base.astype(ml_dtypes.bfloat16), "ident": ident}


_NC_CACHE = None
LAST_EXEC_NS = None


def kernel(x, w_offset, w_deform):
    global _NC_CACHE
    x = np.asarray(x, np.float32)
    w_offset = np.asarray(w_offset, np.float32)
    w_deform = np.asarray(w_deform, np.float32)

    consts = _host_inputs(w_offset, w_deform)
    in_maps = [dict(consts, x=np.ascontiguousarray(x[i])) for i in range(B)]

    if _NC_CACHE is None:
        _NC_CACHE = _build_nc()
    nc = _NC_CACHE

    from concourse.bass_utils import run_bass_kernel_spmd
    global LAST_EXEC_NS
    trace = bool(os.environ.get("BASS_TRACE"))
    res = run_bass_kernel_spmd(nc, in_maps, core_ids=list(range(NCORES)),
                               trace=trace)
    LAST_EXEC_NS = res.exec_time_ns
    return np.stack([res.results[i]["out"] for i in range(B)], axis=0)


if __name__ == "__main__":
    import jax
    import reference
    cpu = jax.devices("cpu")[0]
    with jax.default_device(cpu):
        jinputs = reference.setup_inputs()
        jexpected = reference.reference(**jinputs)
    inputs = {k: np.asarray(jax.device_get(v)) for k, v in jinputs.items()}
    expected = np.asarray(jax.device_get(jexpected))
    actual = kernel(**inputs)
    rel = np.linalg.norm(actual - expected) / np.linalg.norm(expected)
    print("Relative error:", rel)
    print("max abs diff:", np.abs(actual - expected).max())
    from concourse.timeline_sim import TimelineSim
    print("HW exec time:", round(TimelineSim(_NC_CACHE).simulate()), "ns")


# revision 22
# speedup vs baseline: 1.5059x; 1.0139x over previous
"""Deformable depthwise conv (DConv) Trainium2 kernel — V4.

V3 -> V4 changes, all aimed at the three measured bottlenecks (Pool 235us
busy incl. 25us of gating-product TTs, 49us serial preamble head, 17us
tail):

- Quarter-image pipelining: the image is processed in 4 row-blocks (qb) of
  16 rows / 1024 px. Each qb's preamble (conv, fields, transposes, index
  build, gating rows) is emitted between the previous blocks' main loops,
  so the first gather starts after ~1/4 of the preamble and the tail
  shrinks to one quarter-block drain.
- Pool engine runs ONLY ap_gathers (205us): the gating-product
  tensor_tensors moved to DVE, and gathers are batched 3 (tap,yc)-units
  per ISA call (num_idxs=3072).
- DVE gating multiplies batched to one TT per gather call ([128, 6144]
  bf16, 2x mode) to amortize instruction overheads.
- Incremental odd-plane build per 16-row load so gathers never wait on
  the full-image shifted copy.

Same math as V3: offsets via 18-row conv on PE; bilinear fields with the
+7.5 round-to-floor bias; pair-packed f32 gathers from padded even/odd
bf16 planes; per-pixel gating rows broadcast via DMA; depthwise weights
applied by PE as diag-matmul accumulation over (tap, y-corner, x-corner)
into PSUM.
"""

import os
import numpy as np

import concourse.bass as bass
import concourse.bacc as bacc
import concourse.mybir as mybir
import concourse.tile as tile

f32 = mybir.dt.float32
bf16 = mybir.dt.bfloat16
i32 = mybir.dt.int32
i16 = mybir.dt.int16

B, C, H, W = 8, 256, 64, 64
HW = H * W            # 4096
PAD = 2
PW = W + 2 * PAD      # 68
NPIX = PW * PW        # 4624
SH = 38               # padded rows per half-image source (half1 starts row 30)
NPH = PW * SH         # 2584 elements per half plane
HALFN = NPH // 2      # odd-plane offset in f32-pair units (1292)
H1R = 30              # first padded row of half 1
KK = 9                # 3x3 taps
NCORES = 8
FBIAS = 7.5           # bias so HW round-to-nearest cast == floor+8

QB = 4                # row-blocks
PXQ = HW // QB        # 1024 px per block
GQ = 64               # 16-px groups per block (W18 columns)
UPC = 3               # (tap,yc) units per gather call
NCALL = 18 // UPC     # 6 gather calls per (qb, h)
NIDX = UPC * PXQ      # 3072 f32-pair gathers per call

AF = mybir.ActivationFunctionType
ALU = mybir.AluOpType


def _build_nc():
    nc = bacc.Bacc("TRN2", target_bir_lowering=False, debug=False,
                   num_devices=NCORES)
    x_d = nc.dram_tensor("x", [C, H, W], f32, kind="ExternalInput")
    wo_d = nc.dram_tensor("wo", [2, 128, KK, 18], bf16, kind="ExternalInput")
    wdiag_d = nc.dram_tensor("wdiag", [128, 18, 128], bf16,
                             kind="ExternalInput")
    base_d = nc.dram_tensor("base", [18, HW], bf16, kind="ExternalInput")
    ident_d = nc.dram_tensor("ident", [32, 32], bf16, kind="ExternalInput")
    out_d = nc.dram_tensor("out", [C, H, W], f32, kind="ExternalOutput")
    gat_d = nc.dram_tensor("gat_scratch", [QB, 18 * 2 * PXQ], bf16,
                           kind="Internal")

    with tile.TileContext(nc) as tc:
        _kernel(tc, out_d, x_d, wo_d, wdiag_d, base_d, ident_d, gat_d)
    nc.compile()
    return nc


def _kernel(tc, out_d, x_d, wo_d, wdiag_d, base_d, ident_d, gat_d):
    nc = tc.nc

    with tc.tile_pool(name="persist", bufs=1) as persist:
        # ---------------- constants ----------------
        wo_sb = [persist.tile([128, KK, 18], bf16, name=f"wo{h}",
                              tag=f"wo{h}") for h in range(2)]
        for h in range(2):
            nc.sync.dma_start(wo_sb[h][:], wo_d[h])
        wdiag = persist.tile([128, 18, 128], bf16, tag="wdiag")
        nc.sync.dma_start(wdiag[:], wdiag_d[:])
        ident = persist.tile([32, 32], bf16, tag="ident")
        nc.sync.dma_start(ident[:], ident_d[:])

        xp2 = [[persist.tile([128, 2, NPH], bf16, name=f"xp2_{hf}_{h}",
                             tag=f"xp2_{hf}_{h}") for h in range(2)]
               for hf in range(2)]
        idxR = persist.tile([128, QB, KK, 2, GQ], i16, tag="idxR")
        cpool_cm = tc.tile_pool(name="cpool", bufs=1)
        cpool = cpool_cm.__enter__()
        base = cpool.tile([18, HW], bf16, tag="base")
        nc.sync.dma_start(base[:], base_d[:])

        ldpool_cm = tc.tile_pool(name="ldpool", bufs=2)
        ldpool = ldpool_cm.__enter__()
        for hf in range(2):
            for h in range(2):
                # zero only the pad ring (loads overwrite the interior):
                # top pad rows, bottom pad rows, and the wrap band of
                # right+left pad columns; odd plane needs its top span and
                # the final element (everything else is copied from even)
                ev, od = xp2[hf][h][:, 0, :], xp2[hf][h][:, 1, :]
                nc.vector.memset(ev[:, 0:PW * PAD + PAD], 0.0)
                nc.vector.memset(ev[:, PW * (SH - PAD) - PAD:NPH], 0.0)
                band = ev[:, PW - PAD:PW - PAD + PW * (SH - 3)].rearrange(
                    "p (r c) -> p r c", c=PW)[:, :, 0:2 * PAD]
                nc.vector.memset(band, 0.0)
                nc.vector.memset(od[:, 0:PW * PAD], 0.0)
                nc.vector.memset(od[:, PW * (SH - PAD) - PAD - 1:NPH], 0.0)

        def load_q(q):
            # contiguous DMA chunk into staging (16KB runs, full DMA rate),
            # then strided bf16 convert into the padded even plane, then
            # the freshly-available span of the odd (shift-by-1) plane.
            r0, r1 = 16 * q + PAD, 16 * q + 16 + PAD  # padded row span
            for h in range(2):
                xs = ldpool.tile([128, 1024], f32, tag=f"xs{h}")
                nc.sync.dma_start(
                    xs[:],
                    x_d[128 * h:128 * (h + 1),
                        16 * q:16 * (q + 1)].rearrange("c y x -> c (y x)"),
                )
                xsv = xs[:].rearrange("p (y x) -> p y x", y=16, x=W)
                for hf in range(2):
                    base_r = H1R * hf
                    a = max(r0, base_r)
                    b = min(r1, base_r + SH)
                    if a >= b:
                        continue
                    dst = xp2[hf][h][:, 0, :].rearrange(
                        "p (y x) -> p y x", y=SH, x=PW)[
                        :, a - base_r:b - base_r, PAD:PAD + W]
                    nc.scalar.activation(dst, xsv[:, a - r0:b - r0, :],
                                         AF.Copy)
                    # odd plane = even shifted one element, over this span
                    lo = 0 if a == base_r else PW * (a - base_r) - 1
                    hi = (NPH - 1 if b == base_r + SH
                          else PW * (b - base_r) - 1)
                    nc.scalar.activation(xp2[hf][h][:, 1, lo:hi],
                                         xp2[hf][h][:, 0, lo + 1:hi + 1],
                                         AF.Copy)

        fpool_cm = tc.tile_pool(name="fpool", bufs=1)
        fpool = fpool_cm.__enter__()
        fS = fpool.tile([18, HW], bf16, tag="fS")
        omfS = fpool.tile([18, HW], bf16, tag="omfS")
        W18 = fpool.tile([16, 256, 18], bf16, tag="W18")

        psc_cm = tc.tile_pool(name="psc", bufs=2, space=bass.MemorySpace.PSUM)
        psc = psc_cm.__enter__()
        psw_cm = tc.tile_pool(name="psw", bufs=2, space=bass.MemorySpace.PSUM)
        psw = psw_cm.__enter__()
        ftmp_cm = tc.tile_pool(name="ftmp", bufs=2)
        ftmp = ftmp_cm.__enter__()
        ipool_cm = tc.tile_pool(name="ipool", bufs=1)
        ipool = ipool_cm.__enter__()
        gatp_cm = tc.tile_pool(name="gatp", bufs=1)
        gatp = gatp_cm.__enter__()

        xpb3 = [[xp2[hf][h][:, 0, :].rearrange("p (y x) -> p y x",
                                               y=SH, x=PW)
                 for h in range(2)] for hf in range(2)]

        def conv_chunk(n):
            oc = ftmp.tile([18, 512], f32, tag="offs")
            pt = psc.tile([18, 512], f32, tag="convps")
            first = True
            hf = 0 if n < 4 else 1
            for t in range(KK):
                dy, dx = t // 3, t % 3
                for h in range(2):
                    r = (dy + 1) + 8 * n - H1R * hf
                    rhs = xpb3[hf][h][:, r:r + 8,
                                      (dx + 1):(dx + 1) + W]
                    nc.tensor.matmul(pt[:], wo_sb[h][:, t, :], rhs,
                                     start=first,
                                     stop=(t == KK - 1 and h == 1))
                    first = False
            nc.scalar.activation(oc[:], pt[:], AF.Copy)
            return oc

        def fields_chunk(n, oc):
            cs = slice(512 * n, 512 * (n + 1))
            nfi = ftmp.tile([18, 512], i32, tag="nfi")
            nf = ftmp.tile([18, 512], f32, tag="nf")
            bS = ftmp.tile([18, 512], bf16, tag="bS")
            fsub = ftmp.tile([18, 512], f32, tag="fsub")
            nc.vector.tensor_add(oc[:], oc[:], base[:, cs])
            nc.vector.tensor_copy(nfi[:], oc[:])
            nc.vector.tensor_copy(nf[:], nfi[:])
            nc.vector.tensor_tensor(fsub[:], oc[:], nf[:],
                                    ALU.subtract)
            nc.scalar.activation(omfS[:, cs], fsub[:], AF.Copy, bias=0.5,
                                 scale=-1.0)
            nc.scalar.activation(fS[:, cs], fsub[:], AF.Copy, bias=0.5,
                                 scale=1.0)
            nc.scalar.activation(bS[:], nf[:], AF.Copy)
            return bS

        def transp_chunk(n, bS):
            for g4 in range(8):
                pw = psw.tile([16, 4, 18], bf16, tag="wrapps")
                for j in range(4):
                    s = 16 * (4 * g4 + j)
                    nc.tensor.transpose(pw[:, j, :], bS[:, s:s + 16],
                                        ident[0:18, 0:18])
                nc.scalar.activation(
                    W18[:, 32 * n + 4 * g4:32 * n + 4 * g4 + 4, :], pw[:],
                    AF.Copy)

        def idx_chain(qb):
            ss = slice(GQ * qb, GQ * (qb + 1))
            ncl = ipool.tile([16, GQ, 18], bf16, tag="ncl")
            FF0 = ipool.tile([16, GQ, KK], f32, tag="FF0")
            ihf = ipool.tile([16, GQ, KK], f32, tag="ihf")
            ihi = ipool.tile([16, GQ, KK], i32, tag="ihi")
            b1 = 1 if qb >= 2 else 0
            nc.vector.tensor_scalar(ncl[:], W18[:, ss, :], 6.0, 72.0,
                                    ALU.max, ALU.min)
            # per-half y clamp keeps reads inside this half's 38 rows; the
            # legit sample range (|off|<2.5 after the global pad design)
            # stays strictly inside these bounds
            ylo, yhi = (6.0, 42.0) if not b1 else (36.0, 72.0)
            nc.vector.tensor_scalar(ncl[:, :, 0:KK], ncl[:, :, 0:KK],
                                    ylo, yhi, ALU.max, ALU.min)
            nc.vector.scalar_tensor_tensor(FF0[:], ncl[:, :, 0:KK], 68.0,
                                           ncl[:, :, KK:18], ALU.mult,
                                           ALU.add)
            nc.vector.tensor_scalar(ihf[:], FF0[:], 0.5,
                                    -207.25 - 1020.0 * b1,
                                    ALU.mult, ALU.add)
            nc.vector.tensor_copy(ihi[:], ihf[:])
            nc.vector.tensor_copy(ihf[:], ihi[:])
            # FF0 <- HALFN * (par = FF0 - 2*ih - 414 - 2040*b1)
            nc.vector.scalar_tensor_tensor(FF0[:], ihf[:], -2.0, FF0[:],
                                           ALU.mult, ALU.add)
            nc.vector.tensor_scalar(FF0[:], FF0[:], -414.0 - 2040.0 * b1,
                                    float(HALFN), ALU.add, ALU.mult)
            # ihf <- idx = ih + HALF*par
            nc.vector.tensor_tensor(ihf[:], ihf[:], FF0[:], ALU.add)
            for yc in range(2):
                dst = idxR[0:16, qb, :, yc, :].rearrange("p k s -> p s k")
                nc.vector.tensor_scalar(dst, ihf[:], 34.0 * yc, 0.0,
                                        ALU.add, ALU.add)
            for st in (16, 32, 64):
                nc.sync.dma_start(
                    idxR[st:2 * st, qb].rearrange("p a b c -> p (a b c)"),
                    idxR[0:st, qb].rearrange("p a b c -> p (a b c)"))

        def gat_build(qb):
            cs = slice(PXQ * qb, PXQ * (qb + 1))
            xx = gatp.tile([KK, 2 * PXQ], bf16, tag="xx")
            nc.sync.dma_start(xx[:, 0:PXQ], omfS[KK:18, cs])
            nc.sync.dma_start(xx[:, PXQ:2 * PXQ], fS[KK:18, cs])
            p1 = gatp.tile([KK, 2 * PXQ], bf16, tag="p1")
            p2 = gatp.tile([KK, 2 * PXQ], bf16, tag="p2")
            p1v = p1[:].rearrange("p (j two) -> p two j", two=2)
            p2v = p2[:].rearrange("p (j two) -> p two j", two=2)
            for dx in range(2):
                wx = xx[:, PXQ * dx:PXQ * (dx + 1)]
                nc.vector.tensor_tensor(p1v[:, dx, :], omfS[0:KK, cs], wx,
                                        ALU.mult)
                nc.vector.tensor_tensor(p2v[:, dx, :], fS[0:KK, cs], wx,
                                        ALU.mult)
            gv = gat_d[qb].rearrange("(k y j) -> k y j", k=KK, y=2)
            nc.sync.dma_start(gv[:, 0, :], p1[:])
            nc.sync.dma_start(gv[:, 1, :], p2[:])

        srcs = [[xp2[hf][h][:].rearrange("p t f -> p (t f)").bitcast(
            f32).unsqueeze(2) for h in range(2)] for hf in range(2)]

        pso_cm = tc.tile_pool(name="pso", bufs=1, space=bass.MemorySpace.PSUM)
        pso = pso_cm.__enter__()
        rpool_cm = tc.tile_pool(name="rpool", bufs=2)
        rpool = rpool_cm.__enter__()
        gpool_cm = tc.tile_pool(name="gpool", bufs=3)
        gpool = gpool_cm.__enter__()
        opool_cm = tc.tile_pool(name="opool", bufs=1)
        opool = opool_cm.__enter__()

        def main_block(qb, pre_steps):
            ops = [pso.tile([128, PXQ], f32, name=f"outps{qb}_{h}",
                            tag=f"outps{h}") for h in range(2)]
            reps = []
            for c in range(NCALL):
                rt = rpool.tile([128, 2 * NIDX], bf16, tag="rep")
                nc.sync.dma_start(
                    rt[:],
                    gat_d[qb, 2 * NIDX * c:2 * NIDX * (c + 1)].unsqueeze(
                        0).broadcast_to([128, 2 * NIDX]))
                reps.append(rt)
            idxflat = idxR[:, qb].rearrange("p k y s -> p (k y s)")
            ic = NIDX // 16
            pend = []

            def flush_mm():
                for c, h, hj in pend:
                    for i in range(UPC):
                        k = (UPC * c + i) // 2
                        for dx in range(2):
                            for m in range(2):
                                ms = slice(512 * m, 512 * (m + 1))
                                nc.tensor.matmul(
                                    ops[h][:, ms],
                                    wdiag[:, 2 * k + h, :],
                                    hj[:, i, dx, ms],
                                    start=(c == 0 and i == 0 and dx == 0),
                                    stop=(c == NCALL - 1 and i == UPC - 1
                                          and dx == 1),
                                )
                pend.clear()

            for c in range(NCALL):
                for h in range(2):
                    gt = gpool.tile([128, NIDX, 1], f32, tag="G")
                    nc.gpsimd.ap_gather(
                        gt[:], srcs[qb // 2][h],
                        idxflat[:, ic * c:ic * (c + 1)],
                        channels=128, num_elems=NPH, d=1,
                        num_idxs=NIDX)
                    gb = gt[:].rearrange("p f one -> p (f one)").bitcast(bf16)
                    nc.vector.tensor_tensor(gb, gb, reps[c][:], ALU.mult)
                    pend.append((c, h, gb.rearrange(
                        "p (u j two) -> p u two j", u=UPC, two=2)))
                # batch both halves' matmuls (2 x 12) into one PE burst
                # so the tensor engine ramps toward full clock
                flush_mm()
                # interleave next block's preamble steps between gather
                # calls so the DVE queue alternates gating TTs with
                # preamble work and the Pool never waits long on gt bufs
                for step in pre_steps.pop(c, []):
                    step()
            flush_mm()
            for h in range(2):
                osb = opool.tile([128, PXQ], f32, tag="osb")
                nc.scalar.activation(osb[:], ops[h][:], AF.Copy)
                nc.sync.dma_start(
                    out_d[128 * h:128 * (h + 1),
                          16 * qb:16 * (qb + 1)].rearrange(
                        "c y x -> c (y x)"),
                    osb[:])

        def preamble_steps(qb):
            """Next block's preamble as thunks keyed by gather-call slot."""
            st = {}

            def s0():
                if qb == 0:
                    load_q(0)
                    load_q(1)
                elif qb == 1:
                    load_q(2)
                    load_q(3)
                st["oc0"] = conv_chunk(2 * qb)

            def s1():
                st["oc1"] = conv_chunk(2 * qb + 1)
                st["bS0"] = fields_chunk(2 * qb, st["oc0"])

            def s2():
                st["bS1"] = fields_chunk(2 * qb + 1, st["oc1"])
                transp_chunk(2 * qb, st["bS0"])

            def s3():
                transp_chunk(2 * qb + 1, st["bS1"])

            def s4():
                idx_chain(qb)

            def s5():
                gat_build(qb)

            return {0: [s0], 1: [s1], 2: [s2], 3: [s3], 4: [s4], 5: [s5]}

        # software-pipelined emission: block qb+1's preamble is emitted
        # piecewise between block qb's gather calls
        for step in preamble_steps(0).values():
            for s in step:
                s()
        main_block(0, preamble_steps(1))
        main_block(1, preamble_steps(2))
        main_block(2, preamble_steps(3))
        main_block(3, {})

        opool_cm.__exit__(None, None, None)
        gpool_cm.__exit__(None, None, None)
        rpool_cm.__exit__(None, None, None)
        pso_cm.__exit__(None, None, None)
        gatp_cm.__exit__(None, None, None)
        ipool_cm.__exit__(None, None, None)
        ftmp_cm.__exit__(None, None, None)
        psw_cm.__exit__(None, None, None)
        psc_cm.__exit__(None, None, None)
        fpool_cm.__exit__(None, None, None)
        ldpool_cm.__exit__(None, None, None)
        cpool_cm.__exit__(None, None, None)


def _host_inputs(w_offset, w_deform):
    """Build per-core constant inputs (everything except the image)."""
    import ml_dtypes
    wo = np.empty((2, 128, KK, 18), np.float32)
    for h in range(2):
        for t in range(KK):
            ky, kx = t // 3, t % 3
            for m in range(18):
                oc = 2 * m if m < 9 else 2 * (m - 9) + 1
                wo[h, :, t, m] = w_offset[oc, 128 * h:128 * (h + 1), ky, kx]
    wd = w_deform.reshape(C, KK)
    wdiag = np.zeros((128, 18, 128), np.float32)
    for k in range(KK):
        for h in range(2):
            np.fill_diagonal(wdiag[:, 2 * k + h, :],
                             wd[128 * h:128 * (h + 1), k])
    base = np.empty((18, HW), np.float32)
    yy, xx = np.mgrid[0:H, 0:W]
    for k in range(KK):
        ky, kx = k // 3, k % 3
        base[k, :] = (yy + ky - 1).reshape(-1) + FBIAS
        base[9 + k, :] = (xx + kx - 1).reshape(-1) + FBIAS
    ident = np.eye(32, dtype=ml_dtypes.bfloat16)
    return {"wo": wo.astype(ml_dtypes.bfloat16),
            "wdiag": wdiag.astype(ml_dtypes.bfloat16),
            "base": base.astype(ml_dtypes.bfloat16), "ident": ident}


_NC_CACHE = None
LAST_EXEC_NS = None


def kernel(x, w_offset, w_deform):
    global _NC_CACHE
    x = np.asarray(x, np.float32)
    w_offset = np.asarray(w_offset, np.float32)
    w_deform = np.asarray(w_deform, np.float32)

    consts = _host_inputs(w_offset, w_deform)
    in_maps = [dict(consts, x=np.ascontiguousarray(x[i])) for i in range(B)]

    if _NC_CACHE is None:
        _NC_CACHE = _build_nc()
    nc = _NC_CACHE

    from concourse.bass_utils import run_bass_kernel_spmd
    global LAST_EXEC_NS
    trace = bool(os.environ.get("BASS_TRACE"))
    res = run_bass_kernel_spmd(nc, in_maps, core_ids=list(range(NCORES)),
                               trace=trace)
    LAST_EXEC_NS = res.exec_time_ns
    return np.stack([res.results[i]["out"] for i in range(B)], axis=0)


if __name__ == "__main__":
    import jax
    import reference
    cpu = jax.devices("cpu")[0]
    with jax.default_device(cpu):
        jinputs = reference.setup_inputs()
        jexpected = reference.reference(**jinputs)
    inputs = {k: np.asarray(jax.device_get(v)) for k, v in jinputs.items()}
    expected = np.asarray(jax.device_get(jexpected))
    actual = kernel(**inputs)
    rel = np.linalg.norm(actual - expected) / np.linalg.norm(expected)
    print("Relative error:", rel)
    print("max abs diff:", np.abs(actual - expected).max())
    from concourse.timeline_sim import TimelineSim
    print("HW exec time:", round(TimelineSim(_NC_CACHE).simulate()), "ns")


# revision 23
# speedup vs baseline: 1.5341x; 1.0187x over previous
"""Deformable depthwise conv (DConv) Trainium2 kernel — V4.

V3 -> V4 changes, all aimed at the three measured bottlenecks (Pool 235us
busy incl. 25us of gating-product TTs, 49us serial preamble head, 17us
tail):

- Quarter-image pipelining: the image is processed in 4 row-blocks (qb) of
  16 rows / 1024 px. Each qb's preamble (conv, fields, transposes, index
  build, gating rows) is emitted between the previous blocks' main loops,
  so the first gather starts after ~1/4 of the preamble and the tail
  shrinks to one quarter-block drain.
- Pool engine runs ONLY ap_gathers (205us): the gating-product
  tensor_tensors moved to DVE, and gathers are batched 3 (tap,yc)-units
  per ISA call (num_idxs=3072).
- DVE gating multiplies batched to one TT per gather call ([128, 6144]
  bf16, 2x mode) to amortize instruction overheads.
- Incremental odd-plane build per 16-row load so gathers never wait on
  the full-image shifted copy.

Same math as V3: offsets via 18-row conv on PE; bilinear fields with the
+7.5 round-to-floor bias; pair-packed f32 gathers from padded even/odd
bf16 planes; per-pixel gating rows broadcast via DMA; depthwise weights
applied by PE as diag-matmul accumulation over (tap, y-corner, x-corner)
into PSUM.
"""

import os
import numpy as np

import concourse.bass as bass
import concourse.bacc as bacc
import concourse.mybir as mybir
import concourse.tile as tile

f32 = mybir.dt.float32
bf16 = mybir.dt.bfloat16
i32 = mybir.dt.int32
i16 = mybir.dt.int16

B, C, H, W = 8, 256, 64, 64
HW = H * W            # 4096
PAD = 2
PW = W + 2 * PAD      # 68
NPIX = PW * PW        # 4624
SH = 38               # padded rows per half-image source (half1 starts row 30)
NPH = PW * SH         # 2584 elements per half plane
HALFN = NPH // 2      # odd-plane offset in f32-pair units (1292)
H1R = 30              # first padded row of half 1
KK = 9                # 3x3 taps
NCORES = 8
FBIAS = 7.5           # bias so HW round-to-nearest cast == floor+8

QB = 4                # row-blocks
PXQ = HW // QB        # 1024 px per block
GQ = 64               # 16-px groups per block (W18 columns)
UPC = 3               # (tap,yc) units per gather call
NCALL = 18 // UPC     # 6 gather calls per (qb, h)
NIDX = UPC * PXQ      # 3072 f32-pair gathers per call

AF = mybir.ActivationFunctionType
ALU = mybir.AluOpType


def _build_nc():
    nc = bacc.Bacc("TRN2", target_bir_lowering=False, debug=False,
                   num_devices=NCORES)
    x_d = nc.dram_tensor("x", [C, H, W], f32, kind="ExternalInput")
    wo_d = nc.dram_tensor("wo", [2, 128, KK, 18], bf16, kind="ExternalInput")
    wdiag_d = nc.dram_tensor("wdiag", [128, 18, 128], bf16,
                             kind="ExternalInput")
    base_d = nc.dram_tensor("base", [18, HW], bf16, kind="ExternalInput")
    ident_d = nc.dram_tensor("ident", [32, 32], bf16, kind="ExternalInput")
    out_d = nc.dram_tensor("out", [C, H, W], f32, kind="ExternalOutput")
    gat_d = nc.dram_tensor("gat_scratch", [QB, 18 * 2 * PXQ], bf16,
                           kind="Internal")

    with tile.TileContext(nc) as tc:
        _kernel(tc, out_d, x_d, wo_d, wdiag_d, base_d, ident_d, gat_d)
    nc.compile()
    return nc


def _kernel(tc, out_d, x_d, wo_d, wdiag_d, base_d, ident_d, gat_d):
    nc = tc.nc

    with tc.tile_pool(name="persist", bufs=1) as persist:
        # ---------------- constants ----------------
        wo_sb = [persist.tile([128, KK, 18], bf16, name=f"wo{h}",
                              tag=f"wo{h}") for h in range(2)]
        for h in range(2):
            nc.sync.dma_start(wo_sb[h][:], wo_d[h])
        wdiag = persist.tile([128, 18, 128], bf16, tag="wdiag")
        nc.sync.dma_start(wdiag[:], wdiag_d[:])
        ident = persist.tile([32, 32], bf16, tag="ident")
        nc.sync.dma_start(ident[:], ident_d[:])

        xp2 = [[persist.tile([128, 2, NPH], bf16, name=f"xp2_{hf}_{h}",
                             tag=f"xp2_{hf}_{h}") for h in range(2)]
               for hf in range(2)]
        idxR = persist.tile([128, QB, KK, 2, GQ], i16, tag="idxR")
        cpool_cm = tc.tile_pool(name="cpool", bufs=1)
        cpool = cpool_cm.__enter__()
        base = cpool.tile([18, HW], bf16, tag="base")
        nc.sync.dma_start(base[:], base_d[:])

        ldpool_cm = tc.tile_pool(name="ldpool", bufs=2)
        ldpool = ldpool_cm.__enter__()
        for hf in range(2):
            for h in range(2):
                # zero only the pad ring (loads overwrite the interior):
                # top pad rows, bottom pad rows, and the wrap band of
                # right+left pad columns; odd plane needs its top span and
                # the final element (everything else is copied from even)
                ev, od = xp2[hf][h][:, 0, :], xp2[hf][h][:, 1, :]
                nc.vector.memset(ev[:, 0:PW * PAD + PAD], 0.0)
                nc.vector.memset(ev[:, PW * (SH - PAD) - PAD:NPH], 0.0)
                band = ev[:, PW - PAD:PW - PAD + PW * (SH - 3)].rearrange(
                    "p (r c) -> p r c", c=PW)[:, :, 0:2 * PAD]
                nc.vector.memset(band, 0.0)
                nc.vector.memset(od[:, 0:PW * PAD], 0.0)
                nc.vector.memset(od[:, PW * (SH - PAD) - PAD - 1:NPH], 0.0)

        def load_q(q):
            # contiguous DMA chunk into staging (16KB runs, full DMA rate),
            # then strided bf16 convert into the padded even plane, then
            # the freshly-available span of the odd (shift-by-1) plane.
            r0, r1 = 16 * q + PAD, 16 * q + 16 + PAD  # padded row span
            for h in range(2):
                xs = ldpool.tile([128, 1024], f32, tag=f"xs{h}")
                nc.sync.dma_start(
                    xs[:],
                    x_d[128 * h:128 * (h + 1),
                        16 * q:16 * (q + 1)].rearrange("c y x -> c (y x)"),
                )
                xsv = xs[:].rearrange("p (y x) -> p y x", y=16, x=W)
                for hf in range(2):
                    base_r = H1R * hf
                    a = max(r0, base_r)
                    b = min(r1, base_r + SH)
                    if a >= b:
                        continue
                    dst = xp2[hf][h][:, 0, :].rearrange(
                        "p (y x) -> p y x", y=SH, x=PW)[
                        :, a - base_r:b - base_r, PAD:PAD + W]
                    nc.scalar.activation(dst, xsv[:, a - r0:b - r0, :],
                                         AF.Copy)
                    # odd plane = even shifted one element, over this span
                    lo = 0 if a == base_r else PW * (a - base_r) - 1
                    hi = (NPH - 1 if b == base_r + SH
                          else PW * (b - base_r) - 1)
                    nc.scalar.activation(xp2[hf][h][:, 1, lo:hi],
                                         xp2[hf][h][:, 0, lo + 1:hi + 1],
                                         AF.Copy)

        fpool_cm = tc.tile_pool(name="fpool", bufs=1)
        fpool = fpool_cm.__enter__()
        fS = fpool.tile([18, HW], bf16, tag="fS")
        omfS = fpool.tile([18, HW], bf16, tag="omfS")
        W18 = fpool.tile([16, 256, 18], bf16, tag="W18")

        psc_cm = tc.tile_pool(name="psc", bufs=2, space=bass.MemorySpace.PSUM)
        psc = psc_cm.__enter__()
        psw_cm = tc.tile_pool(name="psw", bufs=2, space=bass.MemorySpace.PSUM)
        psw = psw_cm.__enter__()
        ftmp_cm = tc.tile_pool(name="ftmp", bufs=2)
        ftmp = ftmp_cm.__enter__()
        ipool_cm = tc.tile_pool(name="ipool", bufs=1)
        ipool = ipool_cm.__enter__()
        gatp_cm = tc.tile_pool(name="gatp", bufs=1)
        gatp = gatp_cm.__enter__()

        xpb3 = [[xp2[hf][h][:, 0, :].rearrange("p (y x) -> p y x",
                                               y=SH, x=PW)
                 for h in range(2)] for hf in range(2)]

        def conv_chunk(n):
            oc = ftmp.tile([18, 512], f32, tag="offs")
            pt = psc.tile([18, 512], f32, tag="convps")
            first = True
            hf = 0 if n < 4 else 1
            for t in range(KK):
                dy, dx = t // 3, t % 3
                for h in range(2):
                    r = (dy + 1) + 8 * n - H1R * hf
                    rhs = xpb3[hf][h][:, r:r + 8,
                                      (dx + 1):(dx + 1) + W]
                    nc.tensor.matmul(pt[:], wo_sb[h][:, t, :], rhs,
                                     start=first,
                                     stop=(t == KK - 1 and h == 1))
                    first = False
            nc.scalar.activation(oc[:], pt[:], AF.Copy)
            return oc

        def fields_chunk(n, oc):
            cs = slice(512 * n, 512 * (n + 1))
            nfi = ftmp.tile([18, 512], i32, tag="nfi")
            nf = ftmp.tile([18, 512], f32, tag="nf")
            bS = ftmp.tile([18, 512], bf16, tag="bS")
            fsub = ftmp.tile([18, 512], f32, tag="fsub")
            nc.vector.tensor_add(oc[:], oc[:], base[:, cs])
            nc.vector.tensor_copy(nfi[:], oc[:])
            nc.vector.tensor_copy(nf[:], nfi[:])
            nc.vector.tensor_tensor(fsub[:], oc[:], nf[:],
                                    ALU.subtract)
            nc.scalar.activation(omfS[:, cs], fsub[:], AF.Copy, bias=0.5,
                                 scale=-1.0)
            nc.scalar.activation(fS[:, cs], fsub[:], AF.Copy, bias=0.5,
                                 scale=1.0)
            nc.scalar.activation(bS[:], nf[:], AF.Copy)
            return bS

        def transp_chunk(n, bS):
            for g4 in range(8):
                pw = psw.tile([16, 4, 18], bf16, tag="wrapps")
                for j in range(4):
                    s = 16 * (4 * g4 + j)
                    nc.tensor.transpose(pw[:, j, :], bS[:, s:s + 16],
                                        ident[0:18, 0:18])
                nc.scalar.activation(
                    W18[:, 32 * n + 4 * g4:32 * n + 4 * g4 + 4, :], pw[:],
                    AF.Copy)

        def idx_chain(qb):
            ss = slice(GQ * qb, GQ * (qb + 1))
            ncl = ipool.tile([16, GQ, 18], bf16, tag="ncl")
            FF0 = ipool.tile([16, GQ, KK], f32, tag="FF0")
            ihf = ipool.tile([16, GQ, KK], f32, tag="ihf")
            ihi = ipool.tile([16, GQ, KK], i32, tag="ihi")
            b1 = 1 if qb >= 2 else 0
            nc.vector.tensor_scalar(ncl[:], W18[:, ss, :], 6.0, 72.0,
                                    ALU.max, ALU.min)
            # per-half y clamp keeps reads inside this half's 38 rows; the
            # legit sample range (|off|<2.5 after the global pad design)
            # stays strictly inside these bounds
            ylo, yhi = (6.0, 42.0) if not b1 else (36.0, 72.0)
            nc.vector.tensor_scalar(ncl[:, :, 0:KK], ncl[:, :, 0:KK],
                                    ylo, yhi, ALU.max, ALU.min)
            nc.vector.scalar_tensor_tensor(FF0[:], ncl[:, :, 0:KK], 68.0,
                                           ncl[:, :, KK:18], ALU.mult,
                                           ALU.add)
            nc.vector.tensor_scalar(ihf[:], FF0[:], 0.5,
                                    -207.25 - 1020.0 * b1,
                                    ALU.mult, ALU.add)
            nc.vector.tensor_copy(ihi[:], ihf[:])
            nc.vector.tensor_copy(ihf[:], ihi[:])
            # FF0 <- HALFN * (par = FF0 - 2*ih - 414 - 2040*b1)
            nc.vector.scalar_tensor_tensor(FF0[:], ihf[:], -2.0, FF0[:],
                                           ALU.mult, ALU.add)
            nc.vector.tensor_scalar(FF0[:], FF0[:], -414.0 - 2040.0 * b1,
                                    float(HALFN), ALU.add, ALU.mult)
            # ihf <- idx = ih + HALF*par
            nc.vector.tensor_tensor(ihf[:], ihf[:], FF0[:], ALU.add)
            for yc in range(2):
                dst = idxR[0:16, qb, :, yc, :].rearrange("p k s -> p s k")
                nc.vector.tensor_scalar(dst, ihf[:], 34.0 * yc, 0.0,
                                        ALU.add, ALU.add)
            for st in (16, 32, 64):
                nc.sync.dma_start(
                    idxR[st:2 * st, qb].rearrange("p a b c -> p (a b c)"),
                    idxR[0:st, qb].rearrange("p a b c -> p (a b c)"))

        def gat_build(qb):
            cs = slice(PXQ * qb, PXQ * (qb + 1))
            xx = gatp.tile([KK, 2 * PXQ], bf16, tag="xx")
            nc.sync.dma_start(xx[:, 0:PXQ], omfS[KK:18, cs])
            nc.sync.dma_start(xx[:, PXQ:2 * PXQ], fS[KK:18, cs])
            p1 = gatp.tile([KK, 2 * PXQ], bf16, tag="p1")
            p2 = gatp.tile([KK, 2 * PXQ], bf16, tag="p2")
            p1v = p1[:].rearrange("p (j two) -> p two j", two=2)
            p2v = p2[:].rearrange("p (j two) -> p two j", two=2)
            for dx in range(2):
                wx = xx[:, PXQ * dx:PXQ * (dx + 1)]
                nc.vector.tensor_tensor(p1v[:, dx, :], omfS[0:KK, cs], wx,
                                        ALU.mult)
                nc.vector.tensor_tensor(p2v[:, dx, :], fS[0:KK, cs], wx,
                                        ALU.mult)
            gv = gat_d[qb].rearrange("(k y j) -> k y j", k=KK, y=2)
            nc.sync.dma_start(gv[:, 0, :], p1[:])
            nc.sync.dma_start(gv[:, 1, :], p2[:])

        srcs = [[xp2[hf][h][:].rearrange("p t f -> p (t f)").bitcast(
            f32).unsqueeze(2) for h in range(2)] for hf in range(2)]

        pso_cm = tc.tile_pool(name="pso", bufs=1, space=bass.MemorySpace.PSUM)
        pso = pso_cm.__enter__()
        rpool_cm = tc.tile_pool(name="rpool", bufs=2)
        rpool = rpool_cm.__enter__()
        gpool_cm = tc.tile_pool(name="gpool", bufs=3)
        gpool = gpool_cm.__enter__()
        opool_cm = tc.tile_pool(name="opool", bufs=1)
        opool = opool_cm.__enter__()

        def main_block(qb, pre_steps):
            ops = [pso.tile([128, PXQ], f32, name=f"outps{qb}_{h}",
                            tag=f"outps{h}") for h in range(2)]
            reps = []
            for c in range(NCALL):
                rt = rpool.tile([128, 2 * NIDX], bf16, tag="rep")
                nc.sync.dma_start(
                    rt[:],
                    gat_d[qb, 2 * NIDX * c:2 * NIDX * (c + 1)].unsqueeze(
                        0).broadcast_to([128, 2 * NIDX]))
                reps.append(rt)
            idxflat = idxR[:, qb].rearrange("p k y s -> p (k y s)")
            ic = NIDX // 16
            pend = []

            def flush_mm():
                for c, h, hj in pend:
                    for i in range(UPC):
                        k = (UPC * c + i) // 2
                        for dx in range(2):
                            for m in range(2):
                                ms = slice(512 * m, 512 * (m + 1))
                                nc.tensor.matmul(
                                    ops[h][:, ms],
                                    wdiag[:, 2 * k + h, :],
                                    hj[:, i, dx, ms],
                                    start=(c == 0 and i == 0 and dx == 0),
                                    stop=(c == NCALL - 1 and i == UPC - 1
                                          and dx == 1),
                                )
                pend.clear()

            for c in range(NCALL):
                for h in range(2):
                    gt = gpool.tile([128, NIDX, 1], f32, tag="G")
                    nc.gpsimd.ap_gather(
                        gt[:], srcs[qb // 2][h],
                        idxflat[:, ic * c:ic * (c + 1)],
                        channels=128, num_elems=NPH, d=1,
                        num_idxs=NIDX)
                    gb = gt[:].rearrange("p f one -> p (f one)").bitcast(bf16)
                    nc.vector.tensor_tensor(gb, gb, reps[c][:], ALU.mult)
                    pend.append((c, h, gb.rearrange(
                        "p (u j two) -> p u two j", u=UPC, two=2)))
                # batch both halves' matmuls (2 x 12) into one PE burst
                # so the tensor engine ramps toward full clock
                flush_mm()
                # interleave next block's preamble steps between gather
                # calls so the DVE queue alternates gating TTs with
                # preamble work and the Pool never waits long on gt bufs
                for step in pre_steps.pop(c, []):
                    step()
            flush_mm()
            for h in range(2):
                osb = opool.tile([128, PXQ], f32, tag="osb")
                nc.scalar.activation(osb[:], ops[h][:], AF.Copy)
                nc.sync.dma_start(
                    out_d[128 * h:128 * (h + 1),
                          16 * qb:16 * (qb + 1)].rearrange(
                        "c y x -> c (y x)"),
                    osb[:])

        def preamble_steps(qb):
            """Next block's preamble as thunks keyed by gather-call slot."""
            st = {}

            def s0():
                if qb == 0:
                    load_q(0)
                    load_q(1)
                elif qb == 1:
                    load_q(2)
                    load_q(3)
                st["oc0"] = conv_chunk(2 * qb)

            def s1():
                st["oc1"] = conv_chunk(2 * qb + 1)
                st["bS0"] = fields_chunk(2 * qb, st["oc0"])

            def s2():
                st["bS1"] = fields_chunk(2 * qb + 1, st["oc1"])
                transp_chunk(2 * qb, st["bS0"])

            def s3():
                transp_chunk(2 * qb + 1, st["bS1"])

            def s4():
                idx_chain(qb)

            def s5():
                gat_build(qb)

            return {0: [s0], 1: [s1, s2], 2: [s3, s4], 3: [s5]}

        # software-pipelined emission: block qb+1's preamble is emitted
        # piecewise between block qb's gather calls
        for step in preamble_steps(0).values():
            for s in step:
                s()
        main_block(0, preamble_steps(1))
        main_block(1, preamble_steps(2))
        main_block(2, preamble_steps(3))
        main_block(3, {})

        opool_cm.__exit__(None, None, None)
        gpool_cm.__exit__(None, None, None)
        rpool_cm.__exit__(None, None, None)
        pso_cm.__exit__(None, None, None)
        gatp_cm.__exit__(None, None, None)
        ipool_cm.__exit__(None, None, None)
        ftmp_cm.__exit__(None, None, None)
        psw_cm.__exit__(None, None, None)
        psc_cm.__exit__(None, None, None)
        fpool_cm.__exit__(None, None, None)
        ldpool_cm.__exit__(None, None, None)
        cpool_cm.__exit__(None, None, None)


def _host_inputs(w_offset, w_deform):
    """Build per-core constant inputs (everything except the image)."""
    import ml_dtypes
    wo = np.empty((2, 128, KK, 18), np.float32)
    for h in range(2):
        for t in range(KK):
            ky, kx = t // 3, t % 3
            for m in range(18):
                oc = 2 * m if m < 9 else 2 * (m - 9) + 1
                wo[h, :, t, m] = w_offset[oc, 128 * h:128 * (h + 1), ky, kx]
    wd = w_deform.reshape(C, KK)
    wdiag = np.zeros((128, 18, 128), np.float32)
    for k in range(KK):
        for h in range(2):
            np.fill_diagonal(wdiag[:, 2 * k + h, :],
                             wd[128 * h:128 * (h + 1), k])
    base = np.empty((18, HW), np.float32)
    yy, xx = np.mgrid[0:H, 0:W]
    for k in range(KK):
        ky, kx = k // 3, k % 3
        base[k, :] = (yy + ky - 1).reshape(-1) + FBIAS
        base[9 + k, :] = (xx + kx - 1).reshape(-1) + FBIAS
    ident = np.eye(32, dtype=ml_dtypes.bfloat16)
    return {"wo": wo.astype(ml_dtypes.bfloat16),
            "wdiag": wdiag.astype(ml_dtypes.bfloat16),
            "base": base.astype(ml_dtypes.bfloat16), "ident": ident}


_NC_CACHE = None
LAST_EXEC_NS = None


def kernel(x, w_offset, w_deform):
    global _NC_CACHE
    x = np.asarray(x, np.float32)
    w_offset = np.asarray(w_offset, np.float32)
    w_deform = np.asarray(w_deform, np.float32)

    consts = _host_inputs(w_offset, w_deform)
    in_maps = [dict(consts, x=np.ascontiguousarray(x[i])) for i in range(B)]

    if _NC_CACHE is None:
        _NC_CACHE = _build_nc()
    nc = _NC_CACHE

    from concourse.bass_utils import run_bass_kernel_spmd
    global LAST_EXEC_NS
    trace = bool(os.environ.get("BASS_TRACE"))
    res = run_bass_kernel_spmd(nc, in_maps, core_ids=list(range(NCORES)),
                               trace=trace)
    LAST_EXEC_NS = res.exec_time_ns
    return np.stack([res.results[i]["out"] for i in range(B)], axis=0)


if __name__ == "__main__":
    import jax
    import reference
    cpu = jax.devices("cpu")[0]
    with jax.default_device(cpu):
        jinputs = reference.setup_inputs()
        jexpected = reference.reference(**jinputs)
    inputs = {k: np.asarray(jax.device_get(v)) for k, v in jinputs.items()}
    expected = np.asarray(jax.device_get(jexpected))
    actual = kernel(**inputs)
    rel = np.linalg.norm(actual - expected) / np.linalg.norm(expected)
    print("Relative error:", rel)
    print("max abs diff:", np.abs(actual - expected).max())
    from concourse.timeline_sim import TimelineSim
    print("HW exec time:", round(TimelineSim(_NC_CACHE).simulate()), "ns")
